# revision 1
# baseline (speedup 1.0000x reference)
"""FAVOR+ (Performer) multi-head causal attention — Trainium2 Bass kernel.

Sharding: 8 cores = 4 batches x 2 head-groups (4 heads each).
Math note: the softmax-kernel stabilizers (stab) and the +eps term only
rescale qp/kp per (l,h) [or globally], which cancels in num/den up to an
O(eps * e^stab) ~ 3e-4 relative perturbation (verified numerically).
We therefore compute raw exp(dd) for Q (diag_q also cancels per-l) and
exp(dd - diag_k) for K, with no eps and no stabilizers -> no collectives.

All matmuls run as float32r (full-rate fp32 streaming mode, needs N>=256
and engine-rounded operands).

Per-core pipeline:
  1. qT,kT = proj(x) in [c=256, L] layout (+bias), v in chunked [L, c]
     layout with an appended ones column per head (fused den/s_k rows).
  2. Per head: qpT/kpT [m, L] feature maps (diag_k via augmented matmul),
     kp_lc [L, m] copy for the state update (diag_k via Exp bias).
  3. Causal chunked scan, C=256: A^T blocks, masked; numden^T[65, C] =
     v_aug.T @ A^T + S_aug.T @ qpT_c ; attnT = num * bcast(1/den);
     S_aug += kp_lc.T @ v_aug.
  4. outT[512, L] = wo.T @ attnT (partial; host sums the 2 head-groups).
"""
import numpy as np

B, L, DIM, H, DK, M = 4, 2048, 512, 8, 64, 256
HPC = 4            # heads per core
CW = 256           # scan chunk width
NC2 = L // 128     # 16
NCC = L // CW      # 8
LT = 512
NLT = L // LT

_COMPILED = None


def _build():
    import concourse.bacc as bacc
    import concourse.mybir as mybir
    from concourse.tile import TileContext

    f32 = mybir.dt.float32
    f32r = mybir.dt.float32r
    EXP = mybir.ActivationFunctionType.Exp

    nc = bacc.Bacc("TRN2", target_bir_lowering=False, debug=False,
                   enable_asserts=False, num_devices=8)

    def din(name, shape):
        return nc.dram_tensor(name, shape, f32, kind="ExternalInput").ap()

    xq = din("xq", [512, L])
    xk = din("xk", [512, L])
    xv = din("xv", [512, L])
    wq = din("wq", [512, 256])
    wk = din("wk", [512, 256])
    wv = din("wv", [513, 256])      # [Wv_slice.T ; bv]
    bq2 = din("bq2", [128, 2])
    bk2 = din("bk2", [128, 2])
    projT = din("projT", [128, 256])  # proj.T stacked twice
    bd = din("bd", [128, 8])
    msk = din("msk", [128, 512])
    wo = din("wo", [256, 512])
    outT = nc.dram_tensor("outT", [512, L], f32, kind="ExternalOutput").ap()
    ksr_d = nc.dram_tensor("ksr_d", [1, HPC * L], f32, kind="Internal").ap()

    with TileContext(nc) as tc:
        with (
            tc.tile_pool(name="const", bufs=1) as cpool,
            tc.tile_pool(name="persist", bufs=1) as ppool,
            tc.tile_pool(name="psF", bufs=2, space="PSUM") as psF,
        ):
            # ---- constants (DMA to f32 staging, engine-round into f32r) ----
            def ldconst(name, shape, src):
                tmp = cpool.tile(shape, f32, name=name + "_s", tag="cstage",
                                 bufs=2)
                nc.sync.dma_start(tmp[:, :], src)
                t = cpool.tile(shape, f32r, name=name, tag=name)
                nc.any.tensor_copy(t[:, :], tmp[:, :])
                return t

            c_projT = ldconst("projT", [128, 256], projT)
            c_bd = ldconst("bd", [128, 8], bd)
            c_wq = [ldconst(f"wq{i}", [128, 256], wq[128 * i:128 * (i + 1), :])
                    for i in range(4)]
            c_wk = [ldconst(f"wk{i}", [128, 256], wk[128 * i:128 * (i + 1), :])
                    for i in range(4)]
            c_wv = [ldconst(f"wv{i}", [128, 256], wv[128 * i:128 * (i + 1), :])
                    for i in range(4)]
            c_wvb = ldconst("wvb", [1, 256], wv[512:513, :])
            c_wo = [ldconst(f"wo{i}", [64, 512], wo[64 * i:64 * (i + 1), :])
                    for i in range(4)]
            c_msk = cpool.tile([128, 512], f32, tag="msk")
            nc.sync.dma_start(c_msk[:, :], msk)
            c_bq = cpool.tile([128, 2], f32, tag="bq")
            nc.sync.dma_start(c_bq[:, :], bq2)
            c_bk = cpool.tile([128, 2], f32, tag="bk")
            nc.sync.dma_start(c_bk[:, :], bk2)
            c_ones32 = cpool.tile([128, 128], f32, tag="ones32")
            nc.any.memset(c_ones32[:, :], 1.0)
            c_ones = cpool.tile([128, 128], f32r, tag="ones")
            nc.any.tensor_copy(c_ones[:, :], c_ones32[:, :])
            c_zero32 = cpool.tile([128, 132], f32, tag="zero32")
            nc.any.memset(c_zero32[:, :], 0.0)

            # persistent activations
            t_qT = [ppool.tile([128, L], f32r, name=f"qT{i}", tag=f"qT{i}")
                    for i in range(2)]
            t_kT = [ppool.tile([128, L], f32r, name=f"kT{i}", tag=f"kT{i}")
                    for i in range(2)]
            t_v = ppool.tile([128, NC2 * 264], f32r, tag="vall")
            t_ksc = ppool.tile([128, NC2 * 4], f32, tag="ksc")

            # ---- Phase 1: projections (x staged + rounded, then released) ----
            xin = tc.tile_pool(name="xin", bufs=1)
            xpool = xin.__enter__()
            t_x = {}
            for nm, src in (("q", xq), ("k", xk), ("v", xv)):
                for i in range(4):
                    tmp = xpool.tile([128, L], f32, name=f"xs{nm}{i}",
                                     tag="xstage", bufs=2)
                    nc.sync.dma_start(tmp[:, :], src[128 * i:128 * (i + 1), :])
                    xt = xpool.tile([128, L], f32r, name=f"x{nm}{i}",
                                    tag=f"x{nm}{i}")
                    nc.any.tensor_copy(xt[:, :], tmp[:, :])
                    t_x[(nm, i)] = xt

            for half in range(2):
                for lt in range(NLT):
                    ls = slice(lt * LT, (lt + 1) * LT)
                    for (wgt, nm, dst, bias) in ((c_wq, "q", t_qT, c_bq),
                                                 (c_wk, "k", t_kT, c_bk)):
                        ps = psF.tile([128, LT], f32, tag="psF")
                        for kt in range(4):
                            nc.tensor.matmul(
                                ps[:, :],
                                wgt[kt][:, 128 * half:128 * (half + 1)],
                                t_x[(nm, kt)][:, ls],
                                start=(kt == 0), stop=(kt == 3))
                        nc.vector.tensor_scalar_add(
                            dst[half][:, ls], ps[:, :], bias[:, half:half + 1])

            v_r = t_v[:, :].rearrange("p (c x) -> p c x", x=66)
            nc.any.tensor_copy(v_r[:, :, 64:66], c_ones32[:, 0:128])
            for ch in range(NC2):
                cs = slice(ch * 128, (ch + 1) * 128)
                ps = psF.tile([128, 256], f32, tag="psF")
                for kt in range(4):
                    nc.tensor.matmul(ps[:, :], t_x[("v", kt)][:, cs],
                                     c_wv[kt][:, :],
                                     start=(kt == 0), stop=False)
                nc.tensor.matmul(ps[:, :], c_ones[0:1, 0:128],
                                 c_wvb[:, :], start=False, stop=True)
                for h in range(HPC):
                    nc.any.tensor_copy(
                        t_v[:, ch * 264 + h * 66:ch * 264 + h * 66 + 64],
                        ps[:, 64 * h:64 * (h + 1)])
            xin.__exit__(None, None, None)

            # ---- Phase 1.5: -diag_k (row layout -> DRAM, column layout) ----
            sqx = tc.tile_pool(name="sqx", bufs=1)
            sqpool = sqx.__enter__()
            t_sq = [sqpool.tile([128, L], f32r, name=f"sq{i}", tag=f"sq{i}")
                    for i in range(2)]
            for half in range(2):
                nc.vector.tensor_mul(t_sq[half][:, :], t_kT[half][:, :],
                                     t_kT[half][:, :])
            for h4 in range(HPC):
                for lt in range(NLT):
                    ls = slice(lt * LT, (lt + 1) * LT)
                    ps = psF.tile([1, LT], f32, tag="psF")
                    for half in range(2):
                        nc.tensor.matmul(
                            ps[:, :],
                            c_bd[:, 4 * half + h4:4 * half + h4 + 1],
                            t_sq[half][:, ls],
                            start=(half == 0), stop=(half == 1))
                    t_ksrt = sqpool.tile([1, LT], f32, tag="ksrt", bufs=2)
                    nc.any.tensor_copy(t_ksrt[:, :], ps[:, :])
                    nc.sync.dma_start(
                        ksr_d[0:1, h4 * L + lt * LT:h4 * L + (lt + 1) * LT],
                        t_ksrt[:, :])
            for ch in range(NC2):
                cs = slice(ch * 128, (ch + 1) * 128)
                ps = psF.tile([128, 4], f32, tag="psF")
                for half in range(2):
                    nc.tensor.matmul(ps[:, :], t_sq[half][:, cs],
                                     c_bd[:, 4 * half:4 * (half + 1)],
                                     start=(half == 0), stop=(half == 1))
                nc.any.tensor_copy(t_ksc[:, 4 * ch:4 * (ch + 1)], ps[:, :])
            sqx.__exit__(None, None, None)

            # ---- Phase 2+3: per head ----
            actx = tc.tile_pool(name="attn", bufs=1)
            apool = actx.__enter__()
            hctx = (tc.tile_pool(name="headbuf", bufs=2),
                    tc.tile_pool(name="headbuf1", bufs=1),
                    tc.tile_pool(name="work", bufs=3),
                    tc.tile_pool(name="psScan", bufs=1, space="PSUM"),
                    tc.tile_pool(name="psND", bufs=2, space="PSUM"))
            hpool, h1pool, wpool, psS_pool, psND_pool = [
                c.__enter__() for c in hctx]
            t_attnT = [apool.tile([64, L], f32r, name=f"attnT{i}",
                                  tag=f"attnT{i}") for i in range(4)]
            for h in range(HPC):
                hh = h // 2
                hr = slice(64 * (h % 2), 64 * (h % 2) + 64)
                pr = slice(64 * (h % 2), 64 * (h % 2) + 64)
                t_qp = [hpool.tile([128, L], f32r, name=f"qp{i}", tag=f"qp{i}")
                        for i in range(2)]
                t_kp = [h1pool.tile([128, L], f32r, name=f"kp{i}",
                                    tag=f"kp{i}") for i in range(2)]
                for lt in range(NLT):
                    ls = slice(lt * LT, (lt + 1) * LT)
                    t_ksrh0 = hpool.tile([1, LT], f32, tag="ksrh0")
                    nc.sync.dma_start(
                        t_ksrh0[:, :],
                        ksr_d[0:1, h * L + lt * LT:h * L + (lt + 1) * LT])
                    t_ksrh = hpool.tile([1, LT], f32r, tag="ksrh")
                    nc.any.tensor_copy(t_ksrh[:, :], t_ksrh0[:, :])
                    for half in range(2):
                        mh = slice(128 * half, 128 * (half + 1))
                        ps = psF.tile([128, LT], f32, tag="psF")
                        nc.tensor.matmul(ps[:, :], c_projT[pr, mh],
                                         t_qT[hh][hr, ls],
                                         start=True, stop=True)
                        nc.scalar.activation(t_qp[half][:, ls], ps[:, :], EXP)
                        ps2 = psF.tile([128, LT], f32, tag="psF")
                        nc.tensor.matmul(ps2[:, :], c_projT[pr, mh],
                                         t_kT[hh][hr, ls],
                                         start=True, stop=False)
                        nc.tensor.matmul(ps2[:, :], c_ones[0:1, 0:128],
                                         t_ksrh[0:1, :],
                                         start=False, stop=True)
                        nc.scalar.activation(t_kp[half][:, ls], ps2[:, :], EXP)
                t_kplc = h1pool.tile([128, NC2 * 256], f32r, tag="kplc")
                for ch in range(NC2):
                    cs = slice(ch * 128, (ch + 1) * 128)
                    ps = psF.tile([128, 256], f32, tag="psF")
                    nc.tensor.matmul(ps[:, :], t_kT[hh][hr, cs],
                                     c_projT[pr, :], start=True, stop=True)
                    nc.scalar.activation(
                        t_kplc[:, 256 * ch:256 * (ch + 1)], ps[:, :], EXP,
                        bias=t_ksc[:, 4 * ch + h:4 * ch + h + 1])

                # scan
                t_S = h1pool.tile([128, 132], f32r, tag="S")
                nc.any.tensor_copy(t_S[:, :], c_zero32[:, :])
                t_den = h1pool.tile([1, L], f32, tag="den")
                for cc in range(NCC):
                    qs = slice(cc * CW, (cc + 1) * CW)
                    ts0 = slice(cc * CW, cc * CW + 128)
                    ts1 = slice(cc * CW + 128, (cc + 1) * CW)
                    psA = psS_pool.tile([128, 512], f32, tag="psA", bufs=2)
                    nc.tensor.matmul(psA[:, 0:256], t_kp[0][:, ts0],
                                     t_qp[0][:, qs], start=True, stop=False)
                    nc.tensor.matmul(psA[:, 0:256], t_kp[1][:, ts0],
                                     t_qp[1][:, qs], start=False, stop=False)
                    nc.tensor.matmul(psA[:, 256:512], t_kp[0][:, ts1],
                                     t_qp[0][:, qs], start=False, stop=False)
                    nc.tensor.matmul(psA[:, 256:512], t_kp[1][:, ts1],
                                     t_qp[1][:, qs], start=False, stop=True)
                    atm = wpool.tile([128, 512], f32r, tag="atm")
                    nc.vector.tensor_mul(atm[:, :], psA[:, :], c_msk[:, :])
                    nd = psND_pool.tile([66, CW], f32, tag="psNDt")
                    c128 = cc * 2
                    va0 = t_v[:, c128 * 264 + h * 66:c128 * 264 + h * 66 + 66]
                    va1 = t_v[:, (c128 + 1) * 264 + h * 66:
                              (c128 + 1) * 264 + h * 66 + 66]
                    nc.tensor.matmul(nd[:, :], va0, atm[:, 0:256],
                                     start=True, stop=False)
                    nc.tensor.matmul(nd[:, :], va1, atm[:, 256:512],
                                     start=False, stop=False)
                    nc.tensor.matmul(nd[:, :], t_S[:, 0:66], t_qp[0][:, qs],
                                     start=False, stop=False)
                    nc.tensor.matmul(nd[:, :], t_S[:, 66:132], t_qp[1][:, qs],
                                     start=False, stop=True)
                    nc.any.tensor_copy(t_attnT[h][:, qs], nd[0:64, :])
                    nc.any.tensor_copy(t_den[0:1, qs], nd[64:65, :])
                    psS = psS_pool.tile([128, 132], f32, tag="psS")
                    nc.tensor.matmul(
                        psS[:, 0:66],
                        t_kplc[:, c128 * 256:c128 * 256 + 128],
                        va0, start=True, stop=False)
                    nc.tensor.matmul(
                        psS[:, 0:66],
                        t_kplc[:, (c128 + 1) * 256:(c128 + 1) * 256 + 128],
                        va1, start=False, stop=False)
                    nc.tensor.matmul(
                        psS[:, 66:132],
                        t_kplc[:, c128 * 256 + 128:c128 * 256 + 256],
                        va0, start=False, stop=False)
                    nc.tensor.matmul(
                        psS[:, 66:132],
                        t_kplc[:, (c128 + 1) * 256 + 128:(c128 + 2) * 256],
                        va1, start=False, stop=True)
                    with nc.allow_low_precision(reason="f32r state accumulate (TF32-rounding ~1e-3, validated vs reference)"):
                        nc.vector.tensor_add(t_S[:, :], t_S[:, :], psS[:, :])
                # division for the whole head, off the chunk chain
                t_rcpr = h1pool.tile([1, L], f32r, tag="rcpr")
                with nc.allow_low_precision(reason="f32r reciprocal for matmul broadcast (validated vs reference)"):
                    nc.vector.reciprocal(t_rcpr[0:1, :], t_den[0:1, :])
                for lt in range(NLT):
                    ls = slice(lt * LT, (lt + 1) * LT)
                    psB = psF.tile([64, LT], f32, name="psB", tag="psF")
                    nc.tensor.matmul(psB[:, :], c_ones[0:1, 0:64],
                                     t_rcpr[0:1, ls], start=True, stop=True)
                    nc.vector.tensor_mul(t_attnT[h][:, ls], t_attnT[h][:, ls],
                                         psB[:, :])
            for c in reversed(hctx):
                c.__exit__(None, None, None)

            # ---- Phase 4: output projection ----
            octx = tc.tile_pool(name="outp", bufs=2)
            opool = octx.__enter__()
            for osub in range(4):
                os_ = slice(128 * osub, 128 * (osub + 1))
                t_o = opool.tile([128, L], f32, tag="outT")
                for lt in range(NLT):
                    ls = slice(lt * LT, (lt + 1) * LT)
                    ps = psF.tile([128, LT], f32, tag="psF")
                    for h in range(4):
                        nc.tensor.matmul(ps[:, :], c_wo[h][:, os_],
                                         t_attnT[h][:, ls],
                                         start=(h == 0), stop=(h == 3))
                    nc.any.tensor_copy(t_o[:, ls], ps[:, :])
                nc.sync.dma_start(outT[os_, :], t_o[:, :])
            octx.__exit__(None, None, None)
            actx.__exit__(None, None, None)

    nc.compile()
    return nc


def _prep_inputs(query, key, value, Wq, bq, Wk, bk, Wv, bv, Wo, bo, proj):
    s = float(DK) ** -0.25
    tri = (np.arange(128)[:, None] <= np.arange(128)[None, :]).astype(np.float32)
    on = np.ones((128, 128), np.float32)
    zr = np.zeros((128, 128), np.float32)
    msk = np.concatenate([tri, on, zr, tri], axis=1)
    bd = np.zeros((128, 8), np.float32)
    for half in range(2):
        for r in range(128):
            bd[r, 4 * half + (2 * half + r // 64)] = -0.5
    pT = np.ascontiguousarray(proj.T)
    common = {"projT": np.concatenate([pT, pT]), "bd": bd, "msk": msk}
    in_maps = []
    for b in range(B):
        for hg in range(2):
            sl = slice(hg * 256, (hg + 1) * 256)
            Wqs, Wks, Wvs = Wq[sl] * s, Wk[sl] * s, Wv[sl]
            bqs, bks, bvs = bq[sl] * s, bk[sl] * s, bv[sl]
            m = dict(common)
            m["xq"] = np.ascontiguousarray(query[b].T)
            m["xk"] = np.ascontiguousarray(key[b].T)
            m["xv"] = np.ascontiguousarray(value[b].T)
            m["wq"] = np.ascontiguousarray(Wqs.T)
            m["wk"] = np.ascontiguousarray(Wks.T)
            m["wv"] = np.concatenate([Wvs.T, bvs[None, :]])
            m["bq2"] = np.stack([bqs[:128], bqs[128:]], axis=1)
            m["bk2"] = np.stack([bks[:128], bks[128:]], axis=1)
            m["wo"] = np.ascontiguousarray(Wo[:, sl].T)
            in_maps.append({k: np.ascontiguousarray(v, np.float32)
                            for k, v in m.items()})
    return in_maps


def kernel(query, key, value, Wq, bq, Wk, bk, Wv, bv, Wo, bo, proj,
           _trace=False):
    global _COMPILED
    from concourse import bass_utils
    args = [np.asarray(a, np.float32) for a in
            (query, key, value, Wq, bq, Wk, bk, Wv, bv, Wo, bo, proj)]
    if _COMPILED is None:
        _COMPILED = _build()
    in_maps = _prep_inputs(*args)
    res = bass_utils.run_bass_kernel_spmd(
        _COMPILED, in_maps, core_ids=list(range(8)), trace=_trace)
    out = np.empty((B, L, DIM), np.float32)
    bo_ = args[10]
    for b in range(B):
        out[b] = (res.results[2 * b]["outT"].T
                  + res.results[2 * b + 1]["outT"].T + bo_)
    if _trace:
        kernel._last = res
    return out



# revision 5
# speedup vs baseline: 1.3304x; 1.3304x over previous
"""FAVOR+ (Performer) multi-head causal attention — Trainium2 Bass kernel.

Sharding: 8 cores = 4 batches x 2 head-groups (4 heads each); no collectives
(host sums the two head-group partials of w_o per batch).

Math note: the softmax-kernel stabilizers and +eps only rescale qp/kp per
(l,h) [or globally], which cancels in num/den up to ~3e-4 relative (verified
numerically). Additionally exp(dd - diag_k) = exp(dd) * g with
g = exp(-0.5|k|^2) a per-position scalar, so g is folded into the v-aug
tensor (per-partition scalar multiply) instead of biasing the kp features:
kp/kplc are plain exp(dd), and v_aug rows (incl. the fused ones columns that
produce s_k/den) are scaled by g.

Precision: f32r (full-rate fp32 streaming) for projections/features state;
bf16 for the moving operands of the scan state-update matmuls (N=66 would
run at 1/4 rate in f32r), the masked A^T blocks, attnT and w_o. Validated
~2e-3 rel err vs reference (tolerance 2e-2).

Per-core pipeline:
  1. qT,kT = proj(x) [c=256, L] (+bias via DVE move), g = exp(-0.5|k|^2)
     per head, v_aug [L-chunked, 4 heads x 66] = (Wv x + bv | ones) * g
     in bf16.
  2. Per head: qpT/kpT [m, L] = exp(projT qT) bf16; kplc [L, m] bf16.
  3. Causal chunked scan, C=256: A^T blocks masked (DVE); numden^T[66, C] =
     va^T A^T + S^T qpT; per-chunk reciprocal of den row; raw attnT copy
     (Pool); S += kplc^T va (PE) accumulated on Pool.
  4. attnT = raw * bcast(1/den) (bf16); outT[512, L] = wo^T attnT (partial;
     host sums the 2 head-groups + bias).
"""
import numpy as np

B, L, DIM, H, DK, M = 4, 2048, 512, 8, 64, 256
HPC = 4            # heads per core
CW = 256           # scan chunk width
NC2 = L // 128     # 16
NCC = L // CW      # 8
LT = 512
NLT = L // LT

_COMPILED = None


def _build():
    import concourse.bacc as bacc
    import concourse.mybir as mybir
    from concourse.tile import TileContext

    f32 = mybir.dt.float32
    f32r = mybir.dt.float32r
    bf16 = mybir.dt.bfloat16
    EXP = mybir.ActivationFunctionType.Exp

    nc = bacc.Bacc("TRN2", target_bir_lowering=False, debug=False,
                   enable_asserts=False, num_devices=8)

    def din(name, shape, dt=f32r):
        return nc.dram_tensor(name, shape, dt, kind="ExternalInput").ap()

    xq = din("xq", [512, L])
    xk = din("xk", [512, L])
    xv = din("xv", [512, L])
    wq = din("wq", [512, 256])
    wk = din("wk", [512, 256])
    wv = din("wv", [513, 264])      # [Wv_slice.T ; bv] with ones cols
    bq2 = din("bq2", [128, 2], f32)
    bk2 = din("bk2", [128, 2], f32)
    projT = din("projT", [128, 256])  # proj.T stacked twice
    bd = din("bd", [128, 8])
    msk = din("msk", [128, 512], f32)
    wo = din("wo", [256, 512], bf16)
    ones = din("ones", [1, 128])
    outT = nc.dram_tensor("outT", [512, L], f32, kind="ExternalOutput").ap()

    with TileContext(nc) as tc:
        with (
            tc.tile_pool(name="const", bufs=1) as cpool,
            tc.tile_pool(name="persist", bufs=1) as ppool,
        ):
            # ---- constants: DMA straight into f32r/bf16 tiles ----
            def ldconst(name, shape, src, dt=f32r):
                t = cpool.tile(shape, dt, name=name, tag=name)
                nc.sync.dma_start(t[:, :], src)
                return t

            c_projT = ldconst("projT", [128, 256], projT)
            c_bd = ldconst("bd", [128, 8], bd)
            c_wq = [ldconst(f"wq{i}", [128, 256], wq[128 * i:128 * (i + 1), :])
                    for i in range(4)]
            c_wk = [ldconst(f"wk{i}", [128, 256], wk[128 * i:128 * (i + 1), :])
                    for i in range(4)]
            c_wv = [ldconst(f"wv{i}", [128, 264], wv[128 * i:128 * (i + 1), :])
                    for i in range(4)]
            c_wvb = ldconst("wvb", [1, 264], wv[512:513, :])
            c_wo = [ldconst(f"wo{i}", [64, 512], wo[64 * i:64 * (i + 1), :],
                            bf16) for i in range(4)]
            c_msk = ldconst("msk", [128, 512], msk, f32)
            c_bq = ldconst("bq", [128, 2], bq2, f32)
            c_bk = ldconst("bk", [128, 2], bk2, f32)
            c_ones = ldconst("ones", [1, 128], ones)
            c_zero = cpool.tile([128, 132], f32, tag="zero")
            nc.gpsimd.memset(c_zero[:, :], 0.0)

            # persistent activations
            t_qT = [ppool.tile([128, L], f32r, name=f"qT{i}", tag=f"qT{i}")
                    for i in range(2)]
            t_kT = [ppool.tile([128, L], f32r, name=f"kT{i}", tag=f"kT{i}")
                    for i in range(2)]
            t_v = ppool.tile([128, NC2 * 264], bf16, tag="vall")
            t_g = ppool.tile([128, NC2 * 4], f32, tag="gall")

            # ---- Phase 1: projections (x DMA'd straight to f32r) ----
            xin = tc.tile_pool(name="xin", bufs=1)
            xpool = xin.__enter__()
            p1ctx = (tc.tile_pool(name="psP", bufs=2, space="PSUM"),
                     tc.tile_pool(name="psV", bufs=2, space="PSUM"),
                     tc.tile_pool(name="psK", bufs=2, space="PSUM"))
            psP_pool, psV_pool, psK_pool = [c.__enter__() for c in p1ctx]
            t_x = {}
            for nm, src in (("q", xq), ("k", xk), ("v", xv)):
                for i in range(4):
                    xt = xpool.tile([128, L], f32r, name=f"x{nm}{i}",
                                    tag=f"x{nm}{i}")
                    nc.sync.dma_start(xt[:, :], src[128 * i:128 * (i + 1), :])
                    t_x[(nm, i)] = xt

            for half in range(2):
                for lt in range(NLT):
                    ls = slice(lt * LT, (lt + 1) * LT)
                    for (wgt, nm, dst, bias) in ((c_wq, "q", t_qT, c_bq),
                                                 (c_wk, "k", t_kT, c_bk)):
                        ps = psP_pool.tile([128, LT], f32, tag="psP")
                        for kt in range(4):
                            nc.tensor.matmul(
                                ps[:, :],
                                wgt[kt][:, 128 * half:128 * (half + 1)],
                                t_x[(nm, kt)][:, ls],
                                start=(kt == 0), stop=(kt == 3))
                        nc.vector.tensor_scalar_add(
                            dst[half][:, ls], ps[:, :], bias[:, half:half + 1])

            # g = exp(-0.5 |k_h(l)|^2), per head, [128 l, 4 h] per chunk
            sqx = tc.tile_pool(name="sqx", bufs=1)
            sqpool = sqx.__enter__()
            t_sq = [sqpool.tile([128, L], f32r, name=f"sq{i}", tag=f"sq{i}")
                    for i in range(2)]
            for half in range(2):
                nc.vector.tensor_mul(t_sq[half][:, :], t_kT[half][:, :],
                                     t_kT[half][:, :])
            for ch in range(NC2):
                cs = slice(ch * 128, (ch + 1) * 128)
                ps = psK_pool.tile([128, 4], f32, tag="psK")
                for half in range(2):
                    nc.tensor.matmul(ps[:, :], t_sq[half][:, cs],
                                     c_bd[:, 4 * half:4 * (half + 1)],
                                     start=(half == 0), stop=(half == 1))
                nc.scalar.activation(t_g[:, 4 * ch:4 * (ch + 1)], ps[:, :],
                                     EXP)
            sqx.__exit__(None, None, None)

            # v_aug = ((Wv x + bv) | ones) * g  -> bf16, per-head slots
            for ch in range(NC2):
                cs = slice(ch * 128, (ch + 1) * 128)
                ps = psV_pool.tile([128, 264], f32, tag="psV")
                for kt in range(4):
                    nc.tensor.matmul(ps[:, :], t_x[("v", kt)][:, cs],
                                     c_wv[kt][:, :],
                                     start=(kt == 0), stop=False)
                nc.tensor.matmul(ps[:, :], c_ones[0:1, 0:128],
                                 c_wvb[:, :], start=False, stop=True)
                for h in range(HPC):
                    nc.vector.tensor_scalar_mul(
                        t_v[:, ch * 264 + h * 66:ch * 264 + (h + 1) * 66],
                        ps[:, h * 66:(h + 1) * 66],
                        t_g[:, 4 * ch + h:4 * ch + h + 1])
            xin.__exit__(None, None, None)
            for c in reversed(p1ctx):
                c.__exit__(None, None, None)

            # ---- Phase 2+3: per head ----
            actx = tc.tile_pool(name="attn", bufs=1)
            apool = actx.__enter__()
            hctx = (tc.tile_pool(name="headbuf", bufs=2),
                    tc.tile_pool(name="work", bufs=3),
                    tc.tile_pool(name="psQK", bufs=2, space="PSUM"),
                    tc.tile_pool(name="psA", bufs=2, space="PSUM"),
                    tc.tile_pool(name="psND", bufs=2, space="PSUM"),
                    tc.tile_pool(name="psS", bufs=1, space="PSUM"),
                    tc.tile_pool(name="psB", bufs=1, space="PSUM"))
            (hpool, wpool, psQK_pool, psA_pool, psND_pool, psS_pool,
             psB_pool) = [c.__enter__() for c in hctx]
            t_attnT = [apool.tile([64, L], bf16, name=f"attnT{i}",
                                  tag=f"attnT{i}") for i in range(4)]
            for h in range(HPC):
                hh = h // 2
                hr = slice(64 * (h % 2), 64 * (h % 2) + 64)
                pr = hr
                t_qp = [hpool.tile([128, L], f32r, name=f"qp{i}", tag=f"qp{i}")
                        for i in range(2)]
                t_kp = [hpool.tile([128, L], f32r, name=f"kp{i}", tag=f"kp{i}")
                        for i in range(2)]
                t_kplc = hpool.tile([128, NC2 * 256], bf16, tag="kplc")
                t_S = hpool.tile([128, 132], f32r, tag="S")
                t_rcp = hpool.tile([1, L], f32r, tag="rcp")
                t_raw = hpool.tile([64, L], f32, tag="raw")
                for lt in range(NLT):
                    ls = slice(lt * LT, (lt + 1) * LT)
                    for half in range(2):
                        mh = slice(128 * half, 128 * (half + 1))
                        ps = psQK_pool.tile([128, LT], f32, tag="psQK")
                        nc.tensor.matmul(ps[:, :], c_projT[pr, mh],
                                         t_qT[hh][hr, ls],
                                         start=True, stop=True)
                        nc.scalar.activation(t_qp[half][:, ls], ps[:, :], EXP)
                        ps2 = psQK_pool.tile([128, LT], f32, tag="psQK")
                        nc.tensor.matmul(ps2[:, :], c_projT[pr, mh],
                                         t_kT[hh][hr, ls],
                                         start=True, stop=True)
                        nc.scalar.activation(t_kp[half][:, ls], ps2[:, :], EXP)
                for j in range(NCC):  # kplc, two 128-chunks per psum tile
                    cs0 = slice(j * 256, j * 256 + 128)
                    cs1 = slice(j * 256 + 128, (j + 1) * 256)
                    ps = psQK_pool.tile([128, 512], f32, tag="psQK")
                    nc.tensor.matmul(ps[:, 0:256], t_kT[hh][hr, cs0],
                                     c_projT[pr, :], start=True, stop=True)
                    nc.tensor.matmul(ps[:, 256:512], t_kT[hh][hr, cs1],
                                     c_projT[pr, :], start=True, stop=True)
                    nc.scalar.activation(
                        t_kplc[:, 512 * j:512 * (j + 1)], ps[:, :], EXP)

                # scan
                nc.gpsimd.tensor_copy(t_S[:, :], c_zero[:, :])
                for cc in range(NCC):
                    qs = slice(cc * CW, (cc + 1) * CW)
                    ts0 = slice(cc * CW, cc * CW + 128)
                    ts1 = slice(cc * CW + 128, (cc + 1) * CW)
                    psA = psA_pool.tile([128, 512], f32, tag="psA")
                    nc.tensor.matmul(psA[:, 0:256], t_kp[0][:, ts0],
                                     t_qp[0][:, qs], start=True, stop=False)
                    nc.tensor.matmul(psA[:, 0:256], t_kp[1][:, ts0],
                                     t_qp[1][:, qs], start=False, stop=True)
                    nc.tensor.matmul(psA[:, 256:512], t_kp[0][:, ts1],
                                     t_qp[0][:, qs], start=True, stop=False)
                    nc.tensor.matmul(psA[:, 256:512], t_kp[1][:, ts1],
                                     t_qp[1][:, qs], start=False, stop=True)
                    atm = wpool.tile([128, 512], bf16, tag="atm")
                    nc.vector.tensor_mul(atm[:, :], psA[:, :], c_msk[:, :])
                    nd = psND_pool.tile([66, CW], f32, tag="psNDt")
                    c128 = cc * 2
                    va0 = t_v[:, c128 * 264 + h * 66:c128 * 264 + h * 66 + 66]
                    va1 = t_v[:, (c128 + 1) * 264 + h * 66:
                              (c128 + 1) * 264 + h * 66 + 66]
                    nc.tensor.matmul(nd[:, :], va0, atm[:, 0:256],
                                     start=True, stop=False)
                    nc.tensor.matmul(nd[:, :], va1, atm[:, 256:512],
                                     start=False, stop=(cc == 0))
                    if cc > 0:
                        nc.tensor.matmul(nd[:, :], t_S[:, 0:66],
                                         t_qp[0][:, qs],
                                         start=False, stop=False)
                        nc.tensor.matmul(nd[:, :], t_S[:, 66:132],
                                         t_qp[1][:, qs],
                                         start=False, stop=True)
                    with nc.allow_low_precision(reason="f32r reciprocal for matmul broadcast (validated vs reference)"):
                        nc.vector.reciprocal(t_rcp[0:1, qs], nd[64:65, :])
                    nc.scalar.copy(t_raw[:, qs], nd[0:64, :])
                    if cc < NCC - 1:
                        psS = psS_pool.tile([128, 132], f32, tag="psS")
                        nc.tensor.matmul(
                            psS[:, 0:66],
                            t_kplc[:, c128 * 256:c128 * 256 + 128],
                            va0, start=True, stop=False)
                        nc.tensor.matmul(
                            psS[:, 0:66],
                            t_kplc[:, (c128 + 1) * 256:(c128 + 1) * 256 + 128],
                            va1, start=False, stop=True)
                        nc.tensor.matmul(
                            psS[:, 66:132],
                            t_kplc[:, c128 * 256 + 128:c128 * 256 + 256],
                            va0, start=True, stop=False)
                        nc.tensor.matmul(
                            psS[:, 66:132],
                            t_kplc[:, (c128 + 1) * 256 + 128:(c128 + 2) * 256],
                            va1, start=False, stop=True)
                        with nc.allow_low_precision(reason="f32r state accumulate (TF32-rounding ~1e-3, validated vs reference)"):
                            nc.vector.tensor_add(t_S[:, :], t_S[:, :],
                                                 psS[:, :])
                # division for the whole head, off the chunk chain
                for lt in range(NLT):
                    ls = slice(lt * LT, (lt + 1) * LT)
                    psB = psB_pool.tile([64, LT], f32, tag="psB")
                    nc.tensor.matmul(psB[:, :], c_ones[0:1, 0:64],
                                     t_rcp[0:1, ls], start=True, stop=True)
                    nc.vector.tensor_mul(t_attnT[h][:, ls], t_raw[:, ls],
                                         psB[:, :])
            for c in reversed(hctx):
                c.__exit__(None, None, None)

            # ---- Phase 4: output projection ----
            octx = tc.tile_pool(name="outp", bufs=2)
            opool = octx.__enter__()
            psO = tc.tile_pool(name="psO", bufs=2, space="PSUM")
            psO_pool = psO.__enter__()
            for osub in range(4):
                os_ = slice(128 * osub, 128 * (osub + 1))
                t_o = opool.tile([128, L], f32, tag="outT")
                for lt in range(NLT):
                    ls = slice(lt * LT, (lt + 1) * LT)
                    ps = psO_pool.tile([128, LT], f32, tag="psO")
                    for h in range(4):
                        nc.tensor.matmul(ps[:, :], c_wo[h][:, os_],
                                         t_attnT[h][:, ls],
                                         start=(h == 0), stop=(h == 3))
                    nc.vector.tensor_copy(t_o[:, ls], ps[:, :])
                nc.sync.dma_start(outT[os_, :], t_o[:, :])
            psO.__exit__(None, None, None)
            octx.__exit__(None, None, None)
            actx.__exit__(None, None, None)

    nc.compile()
    return nc


def _prep_inputs(query, key, value, Wq, bq, Wk, bk, Wv, bv, Wo, bo, proj):
    import ml_dtypes
    s = float(DK) ** -0.25
    tri = (np.arange(128)[:, None] <= np.arange(128)[None, :]).astype(np.float32)
    on = np.ones((128, 128), np.float32)
    zr = np.zeros((128, 128), np.float32)
    msk = np.concatenate([tri, on, zr, tri], axis=1)
    bd = np.zeros((128, 8), np.float32)
    for half in range(2):
        for r in range(128):
            bd[r, 4 * half + (2 * half + r // 64)] = -0.5
    pT = np.ascontiguousarray(proj.T)
    common = {"projT": np.concatenate([pT, pT]), "bd": bd, "msk": msk,
              "ones": np.ones((1, 128), np.float32)}
    in_maps = []
    for b in range(B):
        for hg in range(2):
            sl = slice(hg * 256, (hg + 1) * 256)
            Wqs, Wks, Wvs = Wq[sl] * s, Wk[sl] * s, Wv[sl]
            bqs, bks, bvs = bq[sl] * s, bk[sl] * s, bv[sl]
            # wv_aug [513, 264]: per head h cols 66h:66h+64 = Wv.T head cols,
            # cols 66h+64:66h+66 zero; bias row: bv at head cols, 1.0 at ones
            wv_aug = np.zeros((513, 264), np.float32)
            for h in range(HPC):
                wv_aug[0:512, 66 * h:66 * h + 64] = Wvs.T[:, 64 * h:64 * (h + 1)]
                wv_aug[512, 66 * h:66 * h + 64] = bvs[64 * h:64 * (h + 1)]
                wv_aug[512, 66 * h + 64:66 * h + 66] = 1.0
            m = dict(common)
            m["xq"] = np.ascontiguousarray(query[b].T)
            m["xk"] = np.ascontiguousarray(key[b].T)
            m["xv"] = np.ascontiguousarray(value[b].T)
            m["wq"] = np.ascontiguousarray(Wqs.T)
            m["wk"] = np.ascontiguousarray(Wks.T)
            m["wv"] = wv_aug
            m["bq2"] = np.stack([bqs[:128], bqs[128:]], axis=1)
            m["bk2"] = np.stack([bks[:128], bks[128:]], axis=1)
            mm = {k: np.ascontiguousarray(v, np.float32) for k, v in m.items()}
            mm["wo"] = np.ascontiguousarray(Wo[:, sl].T).astype(
                ml_dtypes.bfloat16)
            in_maps.append(mm)
    return in_maps


def kernel(query, key, value, Wq, bq, Wk, bk, Wv, bv, Wo, bo, proj,
           _trace=False):
    global _COMPILED
    from concourse import bass_utils
    args = [np.asarray(a, np.float32) for a in
            (query, key, value, Wq, bq, Wk, bk, Wv, bv, Wo, bo, proj)]
    if _COMPILED is None:
        _COMPILED = _build()
    in_maps = _prep_inputs(*args)
    res = bass_utils.run_bass_kernel_spmd(
        _COMPILED, in_maps, core_ids=list(range(8)), trace=_trace)
    out = np.empty((B, L, DIM), np.float32)
    bo_ = args[10]
    for b in range(B):
        out[b] = (res.results[2 * b]["outT"].T
                  + res.results[2 * b + 1]["outT"].T + bo_)
    if _trace:
        kernel._last = res
    return out


# revision 13
# speedup vs baseline: 1.4197x; 1.0671x over previous
"""FAVOR+ (Performer) multi-head causal attention — Trainium2 Bass kernel.

Sharding: 8 cores = 4 batches x 2 head-groups (4 heads each); no collectives
(host sums the two head-group partials of w_o per batch).

Math note: the softmax-kernel stabilizers and +eps only rescale qp/kp per
(l,h) [or globally], which cancels in num/den up to ~3e-4 relative (verified
numerically). Additionally exp(dd - diag_k) = exp(dd) * g with
g = exp(-0.5|k|^2) a per-position scalar, so g is folded into the v-aug
tensor (per-partition scalar multiply) instead of biasing the kp features:
kp/kplc are plain exp(dd), and v_aug rows (incl. the fused ones columns that
produce s_k/den) are scaled by g.

Precision: f32r (full-rate fp32 streaming) for projections/features/state;
bf16 for the moving operands of the scan state-update matmuls (N=66 would
run at 1/4 rate in f32r), the masked A^T blocks, attnT raw/final and w_o.
Validated ~3e-3 rel err vs reference (tolerance 2e-2).

Schedule notes: all PSUM pools are allocated once up front (no phase
barriers); DMAs are issued x-interleaved-with-weights so the first
projection matmul starts ~3us in; head h+1's feature maps (Act-bound)
overlap head h's scan (DVE/PE-bound) via bufs=2 pools; the v_aug production
(DVE-bound) is emitted between head-0 feature maps and head-0 scan so it
fills the DVE while Act runs Exps.
"""
import numpy as np

B, L, DIM, H, DK, M = 4, 2048, 512, 8, 64, 256
HPC = 4            # heads per core
CW = 256           # scan chunk width
NC2 = L // 128     # 16
NCC = L // CW      # 8
LT = 512
NLT = L // LT

_COMPILED = None


def _build():
    import concourse.bacc as bacc
    import concourse.mybir as mybir
    from concourse.tile import TileContext

    f32 = mybir.dt.float32
    f32r = mybir.dt.float32r
    bf16 = mybir.dt.bfloat16
    EXP = mybir.ActivationFunctionType.Exp

    nc = bacc.Bacc("TRN2", target_bir_lowering=False, debug=False,
                   enable_asserts=False, num_devices=8)

    def din(name, shape, dt=f32r):
        return nc.dram_tensor(name, shape, dt, kind="ExternalInput").ap()

    xq = din("xq", [512, L])
    xk = din("xk", [512, L])
    xv = din("xv", [512, L])
    wq = din("wq", [512, 256])
    wk = din("wk", [512, 256])
    wv = din("wv", [513, 264])      # [Wv_slice.T ; bv] with ones cols
    bq2 = din("bq2", [128, 2], f32)
    bk2 = din("bk2", [128, 2], f32)
    projT = din("projT", [128, 256])  # proj.T stacked twice
    bd = din("bd", [128, 8])
    msk = din("msk", [128, 512], f32)
    wo = din("wo", [256, 512], bf16)
    ones = din("ones", [1, 128])
    outT = nc.dram_tensor("outT", [512, L], f32, kind="ExternalOutput").ap()

    with TileContext(nc) as tc:
        with (
            tc.tile_pool(name="const", bufs=1) as cpool,
            tc.tile_pool(name="persist", bufs=1) as ppool,
            tc.tile_pool(name="psQK", bufs=2, space="PSUM") as psQK_pool,
            tc.tile_pool(name="psA", bufs=2, space="PSUM") as psA_pool,
            tc.tile_pool(name="psND", bufs=2, space="PSUM") as psND_pool,
            tc.tile_pool(name="psS", bufs=1, space="PSUM") as psS_pool,
            tc.tile_pool(name="psB", bufs=1, space="PSUM") as psB_pool,
        ):
            def ldconst(name, shape, src, dt=f32r):
                t = cpool.tile(shape, dt, name=name, tag=name)
                nc.sync.dma_start(t[:, :], src)
                return t

            # x pools entered early so their SBUF ranges are independent;
            # xqk (innermost) released right after the q/k projections,
            # then sqx after ksc, then xv after v_aug (LIFO).
            xvin = tc.tile_pool(name="xvin", bufs=1)
            xvpool = xvin.__enter__()
            sqx = tc.tile_pool(name="sqx", bufs=1)
            sqpool = sqx.__enter__()
            xqkin = tc.tile_pool(name="xqkin", bufs=1)
            xqkpool = xqkin.__enter__()

            # ---- DMA issue order: x interleaved with weights ----
            c_wq = [ldconst(f"wq{i}", [128, 256], wq[128 * i:128 * (i + 1), :])
                    for i in range(4)]
            t_x = {}
            for i in range(4):
                xt = xqkpool.tile([128, L], f32r, name=f"xq{i}", tag=f"xq{i}")
                nc.sync.dma_start(xt[:, :], xq[128 * i:128 * (i + 1), :])
                t_x[("q", i)] = xt
            c_wk = [ldconst(f"wk{i}", [128, 256], wk[128 * i:128 * (i + 1), :])
                    for i in range(4)]
            for i in range(4):
                xt = xqkpool.tile([128, L], f32r, name=f"xk{i}", tag=f"xk{i}")
                nc.sync.dma_start(xt[:, :], xk[128 * i:128 * (i + 1), :])
                t_x[("k", i)] = xt
            c_projT = ldconst("projT", [128, 256], projT)
            c_bd = ldconst("bd", [128, 8], bd)
            c_wv = [ldconst(f"wv{i}", [128, 264], wv[128 * i:128 * (i + 1), :])
                    for i in range(4)]
            c_wvb = ldconst("wvb", [1, 264], wv[512:513, :])
            for i in range(4):
                xt = xvpool.tile([128, L], f32r, name=f"xv{i}", tag=f"xv{i}")
                nc.sync.dma_start(xt[:, :], xv[128 * i:128 * (i + 1), :])
                t_x[("v", i)] = xt
            c_msk = ldconst("msk", [128, 512], msk, f32)
            c_bq = ldconst("bq", [128, 2], bq2, f32)
            c_bk = ldconst("bk", [128, 2], bk2, f32)
            c_ones = ldconst("ones", [1, 128], ones)
            c_wo = [ldconst(f"wo{i}", [64, 512], wo[64 * i:64 * (i + 1), :],
                            bf16) for i in range(4)]
            c_zero = cpool.tile([128, 132], f32, tag="zero")
            nc.gpsimd.memset(c_zero[:, :], 0.0)

            # persistent activations
            t_qT = [ppool.tile([128, L], f32r, name=f"qT{i}", tag=f"qT{i}")
                    for i in range(2)]
            t_kT = [ppool.tile([128, L], f32r, name=f"kT{i}", tag=f"kT{i}")
                    for i in range(2)]
            t_v = ppool.tile([128, NC2 * 264], bf16, tag="vall")
            t_g = ppool.tile([128, NC2 * 4], f32, tag="gall")

            # ---- P1b: q/k projections + kT^2 (fine-grained) ----
            t_sq = [sqpool.tile([128, L], f32r, name=f"sq{i}", tag=f"sq{i}")
                    for i in range(2)]
            for half in range(2):
                for lt in range(NLT):
                    ls = slice(lt * LT, (lt + 1) * LT)
                    for (wgt, nm, dst, bias) in ((c_wq, "q", t_qT, c_bq),
                                                 (c_wk, "k", t_kT, c_bk)):
                        ps = psQK_pool.tile([128, LT], f32, tag="psQK")
                        for kt in range(4):
                            nc.tensor.matmul(
                                ps[:, :],
                                wgt[kt][:, 128 * half:128 * (half + 1)],
                                t_x[(nm, kt)][:, ls],
                                start=(kt == 0), stop=(kt == 3))
                        nc.vector.tensor_scalar_add(
                            dst[half][:, ls], ps[:, :], bias[:, half:half + 1])
                    nc.vector.tensor_mul(t_sq[half][:, ls],
                                         t_kT[half][:, ls],
                                         t_kT[half][:, ls])
            xqkin.__exit__(None, None, None)

            # ksc + g = exp(-0.5 |k_h(l)|^2), [128 l, 4 h] per 128-chunk
            for ch in range(NC2):
                cs = slice(ch * 128, (ch + 1) * 128)
                ps = psQK_pool.tile([128, LT], f32, tag="psQK")
                for half in range(2):
                    nc.tensor.matmul(ps[:, 0:4], t_sq[half][:, cs],
                                     c_bd[:, 4 * half:4 * (half + 1)],
                                     start=(half == 0), stop=(half == 1))
                nc.scalar.activation(t_g[:, 4 * ch:4 * (ch + 1)],
                                     ps[:, 0:4], EXP)
            sqx.__exit__(None, None, None)

            # ---- Phase 2+3: per head (v_aug emitted after head-0 maps) ----
            actx = tc.tile_pool(name="attn", bufs=1)
            apool = actx.__enter__()
            hctx = (tc.tile_pool(name="headbuf", bufs=2),
                    tc.tile_pool(name="work", bufs=3))
            hpool, wpool = [c.__enter__() for c in hctx]
            t_attnT = [apool.tile([64, L], bf16, name=f"attnT{i}",
                                  tag=f"attnT{i}") for i in range(4)]
            for h in range(HPC):
                hh = h // 2
                hr = slice(64 * (h % 2), 64 * (h % 2) + 64)
                pr = hr
                t_qp = [hpool.tile([128, L], f32r, name=f"qp{i}", tag=f"qp{i}")
                        for i in range(2)]
                t_kp = [hpool.tile([128, L], f32r, name=f"kp{i}", tag=f"kp{i}",
                                   bufs=1)
                        for i in range(2)]
                t_kplc = hpool.tile([128, NC2 * 256], bf16, tag="kplc")
                t_S = hpool.tile([128, 132], f32r, tag="S")
                t_rcp = hpool.tile([1, L], f32r, tag="rcp", bufs=1)
                t_raw = hpool.tile([64, L], bf16, tag="raw")
                for lt in range(NLT):
                    ls = slice(lt * LT, (lt + 1) * LT)
                    for half in range(2):
                        mh = slice(128 * half, 128 * (half + 1))
                        ps = psQK_pool.tile([128, LT], f32, tag="psQK")
                        nc.tensor.matmul(ps[:, :], c_projT[pr, mh],
                                         t_qT[hh][hr, ls],
                                         start=True, stop=True)
                        nc.scalar.activation(t_qp[half][:, ls], ps[:, :], EXP)
                        ps2 = psQK_pool.tile([128, LT], f32, tag="psQK")
                        nc.tensor.matmul(ps2[:, :], c_projT[pr, mh],
                                         t_kT[hh][hr, ls],
                                         start=True, stop=True)
                        nc.scalar.activation(t_kp[half][:, ls], ps2[:, :], EXP)
                for j in range(NCC):  # kplc, two 128-chunks per psum tile
                    cs0 = slice(j * 256, j * 256 + 128)
                    cs1 = slice(j * 256 + 128, (j + 1) * 256)
                    ps = psQK_pool.tile([128, 512], f32, tag="psQK")
                    nc.tensor.matmul(ps[:, 0:256], t_kT[hh][hr, cs0],
                                     c_projT[pr, :], start=True, stop=True)
                    nc.tensor.matmul(ps[:, 256:512], t_kT[hh][hr, cs1],
                                     c_projT[pr, :], start=True, stop=True)
                    nc.scalar.activation(
                        t_kplc[:, 512 * j:512 * (j + 1)], ps[:, :], EXP)

                if h == 0:
                    # v_aug = ((Wv x + bv) | ones) * g -> bf16 per-head slots.
                    # Emitted here: its PE/DVE work overlaps head-0's Exps.
                    for ch in range(NC2):
                        cs = slice(ch * 128, (ch + 1) * 128)
                        ps = psA_pool.tile([128, 512], f32, tag="psA")
                        for kt in range(4):
                            nc.tensor.matmul(ps[:, 0:264],
                                             t_x[("v", kt)][:, cs],
                                             c_wv[kt][:, :],
                                             start=(kt == 0), stop=False)
                        nc.tensor.matmul(ps[:, 0:264], c_ones[0:1, 0:128],
                                         c_wvb[:, :], start=False, stop=True)
                        for hv in range(HPC):
                            nc.vector.tensor_scalar_mul(
                                t_v[:, ch * 264 + hv * 66:
                                    ch * 264 + (hv + 1) * 66],
                                ps[:, hv * 66:(hv + 1) * 66],
                                t_g[:, 4 * ch + hv:4 * ch + hv + 1])

                # scan
                nc.gpsimd.tensor_copy(t_S[:, :], c_zero[:, :])
                for cc in range(NCC):
                    qs = slice(cc * CW, (cc + 1) * CW)
                    ts0 = slice(cc * CW, cc * CW + 128)
                    ts1 = slice(cc * CW + 128, (cc + 1) * CW)
                    psA = psA_pool.tile([128, 512], f32, tag="psA")
                    nc.tensor.matmul(psA[:, 0:256], t_kp[0][:, ts0],
                                     t_qp[0][:, qs], start=True, stop=False)
                    nc.tensor.matmul(psA[:, 0:256], t_kp[1][:, ts0],
                                     t_qp[1][:, qs], start=False, stop=True)
                    nc.tensor.matmul(psA[:, 256:512], t_kp[0][:, ts1],
                                     t_qp[0][:, qs], start=True, stop=False)
                    nc.tensor.matmul(psA[:, 256:512], t_kp[1][:, ts1],
                                     t_qp[1][:, qs], start=False, stop=True)
                    atm = wpool.tile([128, 512], bf16, tag="atm")
                    nc.vector.tensor_mul(atm[:, :], psA[:, :], c_msk[:, :])
                    nd = psND_pool.tile([66, CW], f32, tag="psNDt")
                    c128 = cc * 2
                    va0 = t_v[:, c128 * 264 + h * 66:c128 * 264 + h * 66 + 66]
                    va1 = t_v[:, (c128 + 1) * 264 + h * 66:
                              (c128 + 1) * 264 + h * 66 + 66]
                    nc.tensor.matmul(nd[:, :], va0, atm[:, 0:256],
                                     start=True, stop=False)
                    nc.tensor.matmul(nd[:, :], va1, atm[:, 256:512],
                                     start=False, stop=(cc == 0))
                    if cc > 0:
                        nc.tensor.matmul(nd[:, :], t_S[:, 0:66],
                                         t_qp[0][:, qs],
                                         start=False, stop=False)
                        nc.tensor.matmul(nd[:, :], t_S[:, 66:132],
                                         t_qp[1][:, qs],
                                         start=False, stop=True)
                    with nc.allow_low_precision(reason="f32r reciprocal for matmul broadcast (validated vs reference)"):
                        nc.vector.reciprocal(t_rcp[0:1, qs], nd[64:65, :])
                    if cc % 2 == 0:
                        nc.scalar.copy(t_raw[:, qs], nd[0:64, :])
                    else:
                        nc.vector.tensor_copy(t_raw[:, qs], nd[0:64, :])
                    if cc < NCC - 1:
                        psS = psS_pool.tile([128, 132], f32, tag="psS")
                        nc.tensor.matmul(
                            psS[:, 0:66],
                            t_kplc[:, c128 * 256:c128 * 256 + 128],
                            va0, start=True, stop=False)
                        nc.tensor.matmul(
                            psS[:, 0:66],
                            t_kplc[:, (c128 + 1) * 256:(c128 + 1) * 256 + 128],
                            va1, start=False, stop=True)
                        nc.tensor.matmul(
                            psS[:, 66:132],
                            t_kplc[:, c128 * 256 + 128:c128 * 256 + 256],
                            va0, start=True, stop=False)
                        nc.tensor.matmul(
                            psS[:, 66:132],
                            t_kplc[:, (c128 + 1) * 256 + 128:(c128 + 2) * 256],
                            va1, start=False, stop=True)
                        with nc.allow_low_precision(reason="f32r state accumulate (TF32-rounding ~1e-3, validated vs reference)"):
                            nc.vector.tensor_add(t_S[:, :], t_S[:, :],
                                                 psS[:, :])
                # division for the whole head, off the chunk chain
                for lt in range(NLT):
                    ls = slice(lt * LT, (lt + 1) * LT)
                    psB = psB_pool.tile([64, LT], f32, tag="psB")
                    nc.tensor.matmul(psB[:, :], c_ones[0:1, 0:64],
                                     t_rcp[0:1, ls], start=True, stop=True)
                    nc.vector.tensor_mul(t_attnT[h][:, ls], t_raw[:, ls],
                                         psB[:, :])
            for c in reversed(hctx):
                c.__exit__(None, None, None)

            # ---- Phase 4: output projection ----
            octx = tc.tile_pool(name="outp", bufs=4)
            opool = octx.__enter__()
            for osub in range(4):
                os_ = slice(128 * osub, 128 * (osub + 1))
                for lt in range(NLT):
                    ls = slice(lt * LT, (lt + 1) * LT)
                    ps = psA_pool.tile([128, 512], f32, tag="psA")
                    for h in range(4):
                        nc.tensor.matmul(ps[:, 0:LT], c_wo[h][:, os_],
                                         t_attnT[h][:, ls],
                                         start=(h == 0), stop=(h == 3))
                    t_o = opool.tile([128, LT], f32, tag="outT")
                    if lt % 2 == 0:
                        nc.scalar.copy(t_o[:, :], ps[:, 0:LT])
                    else:
                        nc.vector.tensor_copy(t_o[:, :], ps[:, 0:LT])
                    nc.sync.dma_start(outT[os_, ls], t_o[:, :])
            octx.__exit__(None, None, None)
            actx.__exit__(None, None, None)
            xvin.__exit__(None, None, None)

    nc.compile()
    return nc


def _prep_inputs(query, key, value, Wq, bq, Wk, bk, Wv, bv, Wo, bo, proj):
    import ml_dtypes
    s = float(DK) ** -0.25
    tri = (np.arange(128)[:, None] <= np.arange(128)[None, :]).astype(np.float32)
    on = np.ones((128, 128), np.float32)
    zr = np.zeros((128, 128), np.float32)
    msk = np.concatenate([tri, on, zr, tri], axis=1)
    bd = np.zeros((128, 8), np.float32)
    for half in range(2):
        for r in range(128):
            bd[r, 4 * half + (2 * half + r // 64)] = -0.5
    pT = np.ascontiguousarray(proj.T)
    common = {"projT": np.concatenate([pT, pT]), "bd": bd, "msk": msk,
              "ones": np.ones((1, 128), np.float32)}
    in_maps = []
    for b in range(B):
        for hg in range(2):
            sl = slice(hg * 256, (hg + 1) * 256)
            Wqs, Wks, Wvs = Wq[sl] * s, Wk[sl] * s, Wv[sl]
            bqs, bks, bvs = bq[sl] * s, bk[sl] * s, bv[sl]
            # wv_aug [513, 264]: per head h cols 66h:66h+64 = Wv.T head cols,
            # cols 66h+64:66h+66 zero; bias row: bv at head cols, 1.0 at ones
            wv_aug = np.zeros((513, 264), np.float32)
            for h in range(HPC):
                wv_aug[0:512, 66 * h:66 * h + 64] = Wvs.T[:, 64 * h:64 * (h + 1)]
                wv_aug[512, 66 * h:66 * h + 64] = bvs[64 * h:64 * (h + 1)]
                wv_aug[512, 66 * h + 64:66 * h + 66] = 1.0
            m = dict(common)
            m["xq"] = np.ascontiguousarray(query[b].T)
            m["xk"] = np.ascontiguousarray(key[b].T)
            m["xv"] = np.ascontiguousarray(value[b].T)
            m["wq"] = np.ascontiguousarray(Wqs.T)
            m["wk"] = np.ascontiguousarray(Wks.T)
            m["wv"] = wv_aug
            m["bq2"] = np.stack([bqs[:128], bqs[128:]], axis=1)
            m["bk2"] = np.stack([bks[:128], bks[128:]], axis=1)
            mm = {k: np.ascontiguousarray(v, np.float32) for k, v in m.items()}
            mm["wo"] = np.ascontiguousarray(Wo[:, sl].T).astype(
                ml_dtypes.bfloat16)
            in_maps.append(mm)
    return in_maps


def kernel(query, key, value, Wq, bq, Wk, bk, Wv, bv, Wo, bo, proj,
           _trace=False):
    global _COMPILED
    from concourse import bass_utils
    args = [np.asarray(a, np.float32) for a in
            (query, key, value, Wq, bq, Wk, bk, Wv, bv, Wo, bo, proj)]
    if _COMPILED is None:
        _COMPILED = _build()
    in_maps = _prep_inputs(*args)
    res = bass_utils.run_bass_kernel_spmd(
        _COMPILED, in_maps, core_ids=list(range(8)), trace=_trace)
    out = np.empty((B, L, DIM), np.float32)
    bo_ = args[10]
    for b in range(B):
        out[b] = (res.results[2 * b]["outT"].T
                  + res.results[2 * b + 1]["outT"].T + bo_)
    if _trace:
        kernel._last = res
    return out


# revision 14
# speedup vs baseline: 1.6560x; 1.1664x over previous
"""FAVOR+ (Performer) multi-head causal attention — Trainium2 Bass kernel.

Sharding: 8 cores = 4 batches x 2 head-groups (4 heads each); no collectives
(host sums the two head-group partials of w_o per batch).

Math note: the softmax-kernel stabilizers and +eps only rescale qp/kp per
(l,h) [or globally], which cancels in num/den up to ~3e-4 relative (verified
numerically). Additionally exp(dd - diag_k) = exp(dd) * g with
g = exp(-0.5|k|^2) a per-position scalar, so g is folded into the v-aug
tensor (per-partition scalar multiply) instead of biasing the kp features:
kp/kplc are plain exp(dd), and v_aug rows (incl. the fused ones columns that
produce s_k/den) are scaled by g.

Precision: f32r (full-rate fp32 streaming) for projections/features/state;
bf16 for the moving operands of the scan state-update matmuls (N=66 would
run at 1/4 rate in f32r), the masked A^T blocks, attnT raw/final and w_o.
Validated ~3e-3 rel err vs reference (tolerance 2e-2).

Schedule notes: all PSUM pools are allocated once up front (no phase
barriers); DMAs are issued x-interleaved-with-weights so the first
projection matmul starts ~3us in; head h+1's feature maps (Act-bound)
overlap head h's scan (DVE/PE-bound) via bufs=2 pools; the v_aug production
(DVE-bound) is emitted between head-0 feature maps and head-0 scan so it
fills the DVE while Act runs Exps.
"""
import numpy as np

B, L, DIM, H, DK, M = 4, 2048, 512, 8, 64, 256
HPC = 4            # heads per core
CW = 256           # scan chunk width
NC2 = L // 128     # 16
NCC = L // CW      # 8
LT = 512
NLT = L // LT

_COMPILED = None


def _build():
    import concourse.bacc as bacc
    import concourse.mybir as mybir
    from concourse.tile import TileContext

    f32 = mybir.dt.float32
    f32r = mybir.dt.float32r
    bf16 = mybir.dt.bfloat16
    EXP = mybir.ActivationFunctionType.Exp

    nc = bacc.Bacc("TRN2", target_bir_lowering=False, debug=False,
                   enable_asserts=False, num_devices=8)

    def din(name, shape, dt=f32r):
        return nc.dram_tensor(name, shape, dt, kind="ExternalInput").ap()

    xq = din("xq", [512, L], bf16)
    xk = din("xk", [512, L], bf16)
    xv = din("xv", [512, L], bf16)
    wq = din("wq", [512, 256], bf16)
    wk = din("wk", [512, 256], bf16)
    wv = din("wv", [513, 264], bf16)  # [Wv_slice.T ; bv] with ones cols
    bq2 = din("bq2", [128, 2], f32)
    bk2 = din("bk2", [128, 2], f32)
    projT = din("projT", [128, 256])  # proj.T stacked twice
    bd = din("bd", [128, 8])
    msk = din("msk", [128, 512], f32)
    wo = din("wo", [256, 512], bf16)
    ones = din("ones", [1, 128])
    ones_bf = din("ones_bf", [1, 128], bf16)
    outT = nc.dram_tensor("outT", [512, L], bf16, kind="ExternalOutput").ap()

    with TileContext(nc) as tc:
        with (
            tc.tile_pool(name="const", bufs=1) as cpool,
            tc.tile_pool(name="persist", bufs=1) as ppool,
            tc.tile_pool(name="psQK", bufs=2, space="PSUM") as psQK_pool,
            tc.tile_pool(name="psA", bufs=2, space="PSUM") as psA_pool,
            tc.tile_pool(name="psND", bufs=2, space="PSUM") as psND_pool,
            tc.tile_pool(name="psS", bufs=1, space="PSUM") as psS_pool,
            tc.tile_pool(name="psB", bufs=1, space="PSUM") as psB_pool,
        ):
            def ldconst(name, shape, src, dt=f32r):
                t = cpool.tile(shape, dt, name=name, tag=name)
                nc.sync.dma_start(t[:, :], src)
                return t

            # x pools entered early so their SBUF ranges are independent;
            # xqk (innermost) released right after the q/k projections,
            # then sqx after ksc, then xv after v_aug (LIFO).
            xvin = tc.tile_pool(name="xvin", bufs=1)
            xvpool = xvin.__enter__()
            sqx = tc.tile_pool(name="sqx", bufs=1)
            sqpool = sqx.__enter__()
            xqkin = tc.tile_pool(name="xqkin", bufs=1)
            xqkpool = xqkin.__enter__()

            # ---- DMA issue order: x interleaved with weights ----
            c_wq = [ldconst(f"wq{i}", [128, 256], wq[128 * i:128 * (i + 1), :],
                            bf16) for i in range(4)]
            t_x = {}
            for i in range(4):
                xt = xqkpool.tile([128, L], bf16, name=f"xq{i}", tag=f"xq{i}")
                nc.sync.dma_start(xt[:, :], xq[128 * i:128 * (i + 1), :])
                t_x[("q", i)] = xt
            c_wk = [ldconst(f"wk{i}", [128, 256], wk[128 * i:128 * (i + 1), :],
                            bf16) for i in range(4)]
            for i in range(4):
                xt = xqkpool.tile([128, L], bf16, name=f"xk{i}", tag=f"xk{i}")
                nc.sync.dma_start(xt[:, :], xk[128 * i:128 * (i + 1), :])
                t_x[("k", i)] = xt
            c_projT = ldconst("projT", [128, 256], projT)
            c_bd = ldconst("bd", [128, 8], bd)
            c_wv = [ldconst(f"wv{i}", [128, 264], wv[128 * i:128 * (i + 1), :],
                            bf16) for i in range(4)]
            c_wvb = ldconst("wvb", [1, 264], wv[512:513, :], bf16)
            for i in range(4):
                xt = xvpool.tile([128, L], bf16, name=f"xv{i}", tag=f"xv{i}")
                nc.sync.dma_start(xt[:, :], xv[128 * i:128 * (i + 1), :])
                t_x[("v", i)] = xt
            c_msk = ldconst("msk", [128, 512], msk, f32)
            c_bq = ldconst("bq", [128, 2], bq2, f32)
            c_bk = ldconst("bk", [128, 2], bk2, f32)
            c_ones = ldconst("ones", [1, 128], ones)
            c_ones_bf = ldconst("ones_bf", [1, 128], ones_bf, bf16)
            c_wo = [ldconst(f"wo{i}", [64, 512], wo[64 * i:64 * (i + 1), :],
                            bf16) for i in range(4)]
            c_zero = cpool.tile([128, 132], f32, tag="zero")
            nc.gpsimd.memset(c_zero[:, :], 0.0)

            # persistent activations
            t_qT = [ppool.tile([128, L], f32r, name=f"qT{i}", tag=f"qT{i}")
                    for i in range(2)]
            t_kT = [ppool.tile([128, L], f32r, name=f"kT{i}", tag=f"kT{i}")
                    for i in range(2)]
            t_v = ppool.tile([128, NC2 * 264], bf16, tag="vall")
            t_g = ppool.tile([128, NC2 * 4], f32, tag="gall")

            # ---- P1b: q/k projections + kT^2 (fine-grained) ----
            t_sq = [sqpool.tile([128, L], f32r, name=f"sq{i}", tag=f"sq{i}")
                    for i in range(2)]
            for half in range(2):
                for lt in range(NLT):
                    ls = slice(lt * LT, (lt + 1) * LT)
                    for (wgt, nm, dst, bias) in ((c_wq, "q", t_qT, c_bq),
                                                 (c_wk, "k", t_kT, c_bk)):
                        ps = psQK_pool.tile([128, LT], f32, tag="psQK")
                        for kt in range(4):
                            nc.tensor.matmul(
                                ps[:, :],
                                wgt[kt][:, 128 * half:128 * (half + 1)],
                                t_x[(nm, kt)][:, ls],
                                start=(kt == 0), stop=(kt == 3))
                        nc.vector.tensor_scalar_add(
                            dst[half][:, ls], ps[:, :], bias[:, half:half + 1])
                    nc.vector.tensor_mul(t_sq[half][:, ls],
                                         t_kT[half][:, ls],
                                         t_kT[half][:, ls])
            xqkin.__exit__(None, None, None)

            # ksc + g = exp(-0.5 |k_h(l)|^2), [128 l, 4 h] per 128-chunk
            for ch in range(NC2):
                cs = slice(ch * 128, (ch + 1) * 128)
                ps = psQK_pool.tile([128, LT], f32, tag="psQK")
                for half in range(2):
                    nc.tensor.matmul(ps[:, 0:4], t_sq[half][:, cs],
                                     c_bd[:, 4 * half:4 * (half + 1)],
                                     start=(half == 0), stop=(half == 1))
                nc.scalar.activation(t_g[:, 4 * ch:4 * (ch + 1)],
                                     ps[:, 0:4], EXP)
            sqx.__exit__(None, None, None)

            # ---- Phase 2+3: per head (v_aug emitted after head-0 maps) ----
            actx = tc.tile_pool(name="attn", bufs=1)
            apool = actx.__enter__()
            hctx = (tc.tile_pool(name="headbuf", bufs=2),
                    tc.tile_pool(name="work", bufs=3))
            hpool, wpool = [c.__enter__() for c in hctx]
            t_attnT = [apool.tile([64, L], bf16, name=f"attnT{i}",
                                  tag=f"attnT{i}") for i in range(4)]
            for h in range(HPC):
                hh = h // 2
                hr = slice(64 * (h % 2), 64 * (h % 2) + 64)
                pr = hr
                t_qp = [hpool.tile([128, L], f32r, name=f"qp{i}", tag=f"qp{i}")
                        for i in range(2)]
                t_kp = [hpool.tile([128, L], f32r, name=f"kp{i}", tag=f"kp{i}")
                        for i in range(2)]
                t_kplc = hpool.tile([128, NC2 * 256], bf16, tag="kplc")
                t_S = hpool.tile([128, 132], f32r, tag="S")
                t_rcp = hpool.tile([1, L], f32r, tag="rcp", bufs=1)
                t_raw = hpool.tile([64, L], bf16, tag="raw")
                for lt in range(NLT):
                    ls = slice(lt * LT, (lt + 1) * LT)
                    for half in range(2):
                        mh = slice(128 * half, 128 * (half + 1))
                        ps = psQK_pool.tile([128, LT], f32, tag="psQK")
                        nc.tensor.matmul(ps[:, :], c_projT[pr, mh],
                                         t_qT[hh][hr, ls],
                                         start=True, stop=True)
                        nc.scalar.activation(t_qp[half][:, ls], ps[:, :], EXP)
                        ps2 = psQK_pool.tile([128, LT], f32, tag="psQK")
                        nc.tensor.matmul(ps2[:, :], c_projT[pr, mh],
                                         t_kT[hh][hr, ls],
                                         start=True, stop=True)
                        nc.scalar.activation(t_kp[half][:, ls], ps2[:, :], EXP)
                for j in range(NCC):  # kplc, two 128-chunks per psum tile
                    cs0 = slice(j * 256, j * 256 + 128)
                    cs1 = slice(j * 256 + 128, (j + 1) * 256)
                    ps = psQK_pool.tile([128, 512], f32, tag="psQK")
                    nc.tensor.matmul(ps[:, 0:256], t_kT[hh][hr, cs0],
                                     c_projT[pr, :], start=True, stop=True)
                    nc.tensor.matmul(ps[:, 256:512], t_kT[hh][hr, cs1],
                                     c_projT[pr, :], start=True, stop=True)
                    nc.scalar.activation(
                        t_kplc[:, 512 * j:512 * (j + 1)], ps[:, :], EXP)

                if h == 0:
                    # v_aug = ((Wv x + bv) | ones) * g -> bf16 per-head slots.
                    # Emitted here: its PE/DVE work overlaps head-0's Exps.
                    for ch in range(NC2):
                        cs = slice(ch * 128, (ch + 1) * 128)
                        ps = psA_pool.tile([128, 512], f32, tag="psA")
                        for kt in range(4):
                            nc.tensor.matmul(ps[:, 0:264],
                                             t_x[("v", kt)][:, cs],
                                             c_wv[kt][:, :],
                                             start=(kt == 0), stop=False)
                        nc.tensor.matmul(ps[:, 0:264], c_ones_bf[0:1, 0:128],
                                         c_wvb[:, :], start=False, stop=True)
                        for hv in range(HPC):
                            nc.vector.tensor_scalar_mul(
                                t_v[:, ch * 264 + hv * 66:
                                    ch * 264 + (hv + 1) * 66],
                                ps[:, hv * 66:(hv + 1) * 66],
                                t_g[:, 4 * ch + hv:4 * ch + hv + 1])

                # scan
                nc.gpsimd.tensor_copy(t_S[:, :], c_zero[:, :])
                for cc in range(NCC):
                    qs = slice(cc * CW, (cc + 1) * CW)
                    ts0 = slice(cc * CW, cc * CW + 128)
                    ts1 = slice(cc * CW + 128, (cc + 1) * CW)
                    psA = psA_pool.tile([128, 512], f32, tag="psA")
                    nc.tensor.matmul(psA[:, 0:256], t_kp[0][:, ts0],
                                     t_qp[0][:, qs], start=True, stop=False)
                    nc.tensor.matmul(psA[:, 0:256], t_kp[1][:, ts0],
                                     t_qp[1][:, qs], start=False, stop=True)
                    nc.tensor.matmul(psA[:, 256:512], t_kp[0][:, ts1],
                                     t_qp[0][:, qs], start=True, stop=False)
                    nc.tensor.matmul(psA[:, 256:512], t_kp[1][:, ts1],
                                     t_qp[1][:, qs], start=False, stop=True)
                    atm = wpool.tile([128, 512], bf16, tag="atm")
                    nc.vector.tensor_mul(atm[:, :], psA[:, :], c_msk[:, :])
                    if cc % 2 == 0:
                        nd2 = psND_pool.tile([66, 2 * CW], f32, tag="psNDt")
                    nd = nd2[:, (cc % 2) * CW:(cc % 2 + 1) * CW]
                    c128 = cc * 2
                    va0 = t_v[:, c128 * 264 + h * 66:c128 * 264 + h * 66 + 66]
                    va1 = t_v[:, (c128 + 1) * 264 + h * 66:
                              (c128 + 1) * 264 + h * 66 + 66]
                    nc.tensor.matmul(nd[:, :], va0, atm[:, 0:256],
                                     start=True, stop=False)
                    nc.tensor.matmul(nd[:, :], va1, atm[:, 256:512],
                                     start=False, stop=(cc == 0))
                    if cc > 0:
                        nc.tensor.matmul(nd[:, :], t_S[:, 0:66],
                                         t_qp[0][:, qs],
                                         start=False, stop=False)
                        nc.tensor.matmul(nd[:, :], t_S[:, 66:132],
                                         t_qp[1][:, qs],
                                         start=False, stop=True)
                    if cc % 2 == 1:
                        ds = slice((cc - 1) * CW, (cc + 1) * CW)
                        with nc.allow_low_precision(reason="f32r reciprocal for matmul broadcast (validated vs reference)"):
                            nc.vector.reciprocal(t_rcp[0:1, ds],
                                                 nd2[64:65, :])
                        if cc % 4 == 1:
                            nc.scalar.copy(t_raw[:, ds], nd2[0:64, :])
                        else:
                            nc.vector.tensor_copy(t_raw[:, ds], nd2[0:64, :])
                    if cc < NCC - 1:
                        psS = psS_pool.tile([128, 132], f32, tag="psS")
                        nc.tensor.matmul(
                            psS[:, 0:66],
                            t_kplc[:, c128 * 256:c128 * 256 + 128],
                            va0, start=True, stop=False)
                        nc.tensor.matmul(
                            psS[:, 0:66],
                            t_kplc[:, (c128 + 1) * 256:(c128 + 1) * 256 + 128],
                            va1, start=False, stop=True)
                        nc.tensor.matmul(
                            psS[:, 66:132],
                            t_kplc[:, c128 * 256 + 128:c128 * 256 + 256],
                            va0, start=True, stop=False)
                        nc.tensor.matmul(
                            psS[:, 66:132],
                            t_kplc[:, (c128 + 1) * 256 + 128:(c128 + 2) * 256],
                            va1, start=False, stop=True)
                        with nc.allow_low_precision(reason="f32r state accumulate (TF32-rounding ~1e-3, validated vs reference)"):
                            nc.vector.tensor_add(t_S[:, :], t_S[:, :],
                                                 psS[:, :])
                # division for the whole head, off the chunk chain
                for lt in range(NLT):
                    ls = slice(lt * LT, (lt + 1) * LT)
                    psB = psB_pool.tile([64, LT], f32, tag="psB")
                    nc.tensor.matmul(psB[:, :], c_ones[0:1, 0:64],
                                     t_rcp[0:1, ls], start=True, stop=True)
                    nc.vector.tensor_mul(t_attnT[h][:, ls], t_raw[:, ls],
                                         psB[:, :])
            for c in reversed(hctx):
                c.__exit__(None, None, None)

            # ---- Phase 4: output projection ----
            octx = tc.tile_pool(name="outp", bufs=4)
            opool = octx.__enter__()
            for lt in range(NLT):
                ls = slice(lt * LT, (lt + 1) * LT)
                for osub in range(4):
                    os_ = slice(128 * osub, 128 * (osub + 1))
                    ps = psA_pool.tile([128, 512], f32, tag="psA")
                    for h in range(4):
                        nc.tensor.matmul(ps[:, 0:LT], c_wo[h][:, os_],
                                         t_attnT[h][:, ls],
                                         start=(h == 0), stop=(h == 3))
                    t_o = opool.tile([128, LT], bf16, tag="outT")
                    if osub % 2 == 0:
                        nc.scalar.copy(t_o[:, :], ps[:, 0:LT])
                    else:
                        nc.vector.tensor_copy(t_o[:, :], ps[:, 0:LT])
                    nc.sync.dma_start(outT[os_, ls], t_o[:, :])
            octx.__exit__(None, None, None)
            actx.__exit__(None, None, None)
            xvin.__exit__(None, None, None)

    nc.compile()
    return nc


def _prep_inputs(query, key, value, Wq, bq, Wk, bk, Wv, bv, Wo, bo, proj):
    import ml_dtypes
    s = float(DK) ** -0.25
    tri = (np.arange(128)[:, None] <= np.arange(128)[None, :]).astype(np.float32)
    on = np.ones((128, 128), np.float32)
    zr = np.zeros((128, 128), np.float32)
    msk = np.concatenate([tri, on, zr, tri], axis=1)
    bd = np.zeros((128, 8), np.float32)
    for half in range(2):
        for r in range(128):
            bd[r, 4 * half + (2 * half + r // 64)] = -0.5
    pT = np.ascontiguousarray(proj.T)
    common = {"projT": np.concatenate([pT, pT]), "bd": bd, "msk": msk,
              "ones": np.ones((1, 128), np.float32),
              "ones_bf": np.ones((1, 128), ml_dtypes.bfloat16)}
    in_maps = []
    for b in range(B):
        for hg in range(2):
            sl = slice(hg * 256, (hg + 1) * 256)
            Wqs, Wks, Wvs = Wq[sl] * s, Wk[sl] * s, Wv[sl]
            bqs, bks, bvs = bq[sl] * s, bk[sl] * s, bv[sl]
            # wv_aug [513, 264]: per head h cols 66h:66h+64 = Wv.T head cols,
            # cols 66h+64:66h+66 zero; bias row: bv at head cols, 1.0 at ones
            wv_aug = np.zeros((513, 264), np.float32)
            for h in range(HPC):
                wv_aug[0:512, 66 * h:66 * h + 64] = Wvs.T[:, 64 * h:64 * (h + 1)]
                wv_aug[512, 66 * h:66 * h + 64] = bvs[64 * h:64 * (h + 1)]
                wv_aug[512, 66 * h + 64:66 * h + 66] = 1.0
            m = dict(common)
            m["xq"] = np.ascontiguousarray(query[b].T)
            m["xk"] = np.ascontiguousarray(key[b].T)
            m["xv"] = np.ascontiguousarray(value[b].T)
            m["wq"] = np.ascontiguousarray(Wqs.T)
            m["wk"] = np.ascontiguousarray(Wks.T)
            m["wv"] = wv_aug
            m["bq2"] = np.stack([bqs[:128], bqs[128:]], axis=1)
            m["bk2"] = np.stack([bks[:128], bks[128:]], axis=1)
            mm = {k: (np.ascontiguousarray(v) if v.dtype == ml_dtypes.bfloat16
                      else np.ascontiguousarray(v, np.float32))
                  for k, v in m.items()}
            for k in ("xq", "xk", "xv", "wq", "wk", "wv"):
                mm[k] = mm[k].astype(ml_dtypes.bfloat16)
            mm["wo"] = np.ascontiguousarray(Wo[:, sl].T).astype(
                ml_dtypes.bfloat16)
            in_maps.append(mm)
    return in_maps


def kernel(query, key, value, Wq, bq, Wk, bk, Wv, bv, Wo, bo, proj,
           _trace=False):
    global _COMPILED
    from concourse import bass_utils
    args = [np.asarray(a, np.float32) for a in
            (query, key, value, Wq, bq, Wk, bk, Wv, bv, Wo, bo, proj)]
    if _COMPILED is None:
        _COMPILED = _build()
    in_maps = _prep_inputs(*args)
    res = bass_utils.run_bass_kernel_spmd(
        _COMPILED, in_maps, core_ids=list(range(8)), trace=_trace)
    out = np.empty((B, L, DIM), np.float32)
    bo_ = args[10]
    for b in range(B):
        out[b] = (res.results[2 * b]["outT"].T.astype(np.float32)
                  + res.results[2 * b + 1]["outT"].T.astype(np.float32)
                  + bo_)
    if _trace:
        kernel._last = res
    return out


# revision 16
# speedup vs baseline: 1.6599x; 1.0024x over previous
"""FAVOR+ (Performer) multi-head causal attention — Trainium2 Bass kernel.

Sharding: 8 cores = 4 batches x 2 head-groups (4 heads each); no collectives
(host sums the two head-group partials of w_o per batch).

Math note: the softmax-kernel stabilizers and +eps only rescale qp/kp per
(l,h) [or globally], which cancels in num/den up to ~3e-4 relative (verified
numerically). Additionally exp(dd - diag_k) = exp(dd) * g with
g = exp(-0.5|k|^2) a per-position scalar, so g is folded into the v-aug
tensor (per-partition scalar multiply) instead of biasing the kp features:
kp/kplc are plain exp(dd), and v_aug rows (incl. the fused ones columns that
produce s_k/den) are scaled by g.

Precision: f32r (full-rate fp32 streaming) for projections/features/state;
bf16 for the moving operands of the scan state-update matmuls (N=66 would
run at 1/4 rate in f32r), the masked A^T blocks, attnT raw/final and w_o.
Validated ~3e-3 rel err vs reference (tolerance 2e-2).

Schedule notes: all PSUM pools are allocated once up front (no phase
barriers); DMAs are issued x-interleaved-with-weights so the first
projection matmul starts ~3us in; head h+1's feature maps (Act-bound)
overlap head h's scan (DVE/PE-bound) via bufs=2 pools; the v_aug production
(DVE-bound) is emitted between head-0 feature maps and head-0 scan so it
fills the DVE while Act runs Exps.
"""
import numpy as np

B, L, DIM, H, DK, M = 4, 2048, 512, 8, 64, 256
HPC = 4            # heads per core
CW = 256           # scan chunk width
NC2 = L // 128     # 16
NCC = L // CW      # 8
LT = 512
NLT = L // LT

_COMPILED = None


def _build():
    import concourse.bacc as bacc
    import concourse.mybir as mybir
    from concourse.tile import TileContext

    f32 = mybir.dt.float32
    f32r = mybir.dt.float32r
    bf16 = mybir.dt.bfloat16
    EXP = mybir.ActivationFunctionType.Exp
    IDN = mybir.ActivationFunctionType.Identity

    nc = bacc.Bacc("TRN2", target_bir_lowering=False, debug=False,
                   enable_asserts=False, num_devices=8)

    def din(name, shape, dt=f32r):
        return nc.dram_tensor(name, shape, dt, kind="ExternalInput").ap()

    xq = din("xq", [512, L], bf16)
    xk = din("xk", [512, L], bf16)
    xv = din("xv", [512, L], bf16)
    wq = din("wq", [512, 256], bf16)
    wk = din("wk", [512, 256], bf16)
    wv = din("wv", [513, 264], bf16)  # [Wv_slice.T ; bv] with ones cols
    bq2 = din("bq2", [128, 2], f32)
    bk2 = din("bk2", [128, 2], f32)
    projT = din("projT", [128, 256])  # proj.T stacked twice
    bd = din("bd", [128, 8])
    msk = din("msk", [128, 512], f32)
    wo = din("wo", [256, 512], bf16)
    ones = din("ones", [1, 128])
    ones_bf = din("ones_bf", [1, 128], bf16)
    outT = nc.dram_tensor("outT", [512, L], bf16, kind="ExternalOutput").ap()

    with TileContext(nc) as tc:
        with (
            tc.tile_pool(name="const", bufs=1) as cpool,
            tc.tile_pool(name="persist", bufs=1) as ppool,
            tc.tile_pool(name="psQK", bufs=2, space="PSUM") as psQK_pool,
            tc.tile_pool(name="psA", bufs=2, space="PSUM") as psA_pool,
            tc.tile_pool(name="psND", bufs=2, space="PSUM") as psND_pool,
            tc.tile_pool(name="psS", bufs=1, space="PSUM") as psS_pool,
            tc.tile_pool(name="psB", bufs=1, space="PSUM") as psB_pool,
        ):
            def ldconst(name, shape, src, dt=f32r):
                t = cpool.tile(shape, dt, name=name, tag=name)
                nc.sync.dma_start(t[:, :], src)
                return t

            # x pools entered early so their SBUF ranges are independent;
            # xqk (innermost) released right after the q/k projections,
            # then sqx after ksc, then xv after v_aug (LIFO).
            xvin = tc.tile_pool(name="xvin", bufs=1)
            xvpool = xvin.__enter__()
            sqx = tc.tile_pool(name="sqx", bufs=1)
            sqpool = sqx.__enter__()
            xqkin = tc.tile_pool(name="xqkin", bufs=1)
            xqkpool = xqkin.__enter__()

            # ---- DMA issue order: x interleaved with weights ----
            c_wq = [ldconst(f"wq{i}", [128, 256], wq[128 * i:128 * (i + 1), :],
                            bf16) for i in range(4)]
            t_x = {}
            for i in range(4):
                xt = xqkpool.tile([128, L], bf16, name=f"xq{i}", tag=f"xq{i}")
                nc.sync.dma_start(xt[:, :], xq[128 * i:128 * (i + 1), :])
                t_x[("q", i)] = xt
            c_wk = [ldconst(f"wk{i}", [128, 256], wk[128 * i:128 * (i + 1), :],
                            bf16) for i in range(4)]
            for i in range(4):
                xt = xqkpool.tile([128, L], bf16, name=f"xk{i}", tag=f"xk{i}")
                nc.sync.dma_start(xt[:, :], xk[128 * i:128 * (i + 1), :])
                t_x[("k", i)] = xt
            c_projT = ldconst("projT", [128, 256], projT)
            c_bd = ldconst("bd", [128, 8], bd)
            c_wv = [ldconst(f"wv{i}", [128, 264], wv[128 * i:128 * (i + 1), :],
                            bf16) for i in range(4)]
            c_wvb = ldconst("wvb", [1, 264], wv[512:513, :], bf16)
            for i in range(4):
                xt = xvpool.tile([128, L], bf16, name=f"xv{i}", tag=f"xv{i}")
                nc.sync.dma_start(xt[:, :], xv[128 * i:128 * (i + 1), :])
                t_x[("v", i)] = xt
            c_msk = ldconst("msk", [128, 512], msk, f32)
            c_bq = ldconst("bq", [128, 2], bq2, f32)
            c_bk = ldconst("bk", [128, 2], bk2, f32)
            c_ones = ldconst("ones", [1, 128], ones)
            c_ones_bf = ldconst("ones_bf", [1, 128], ones_bf, bf16)
            c_wo = [ldconst(f"wo{i}", [64, 512], wo[64 * i:64 * (i + 1), :],
                            bf16) for i in range(4)]
            c_zero = cpool.tile([128, 132], f32, tag="zero")
            nc.gpsimd.memset(c_zero[:, :], 0.0)

            # persistent activations
            t_qT = [ppool.tile([128, L], f32r, name=f"qT{i}", tag=f"qT{i}")
                    for i in range(2)]
            t_kT = [ppool.tile([128, L], f32r, name=f"kT{i}", tag=f"kT{i}")
                    for i in range(2)]
            t_v = ppool.tile([128, NC2 * 264], bf16, tag="vall")
            t_g = ppool.tile([128, NC2 * 4], f32, tag="gall")

            # ---- P1b: q/k projections + kT^2 (fine-grained) ----
            t_sq = [sqpool.tile([128, L], f32r, name=f"sq{i}", tag=f"sq{i}")
                    for i in range(2)]
            for half in range(2):
                for lt in range(NLT):
                    ls = slice(lt * LT, (lt + 1) * LT)
                    for (wgt, nm, dst, bias) in ((c_wq, "q", t_qT, c_bq),
                                                 (c_wk, "k", t_kT, c_bk)):
                        ps = psQK_pool.tile([128, LT], f32, tag="psQK")
                        for kt in range(4):
                            nc.tensor.matmul(
                                ps[:, :],
                                wgt[kt][:, 128 * half:128 * (half + 1)],
                                t_x[(nm, kt)][:, ls],
                                start=(kt == 0), stop=(kt == 3))
                        nc.scalar.activation(
                            dst[half][:, ls], ps[:, :], IDN,
                            bias=bias[:, half:half + 1])
                    nc.vector.tensor_mul(t_sq[half][:, ls],
                                         t_kT[half][:, ls],
                                         t_kT[half][:, ls])
            xqkin.__exit__(None, None, None)

            # ksc + g = exp(-0.5 |k_h(l)|^2), [128 l, 4 h] per 128-chunk
            for ch in range(NC2):
                cs = slice(ch * 128, (ch + 1) * 128)
                ps = psQK_pool.tile([128, LT], f32, tag="psQK")
                for half in range(2):
                    nc.tensor.matmul(ps[:, 0:4], t_sq[half][:, cs],
                                     c_bd[:, 4 * half:4 * (half + 1)],
                                     start=(half == 0), stop=(half == 1))
                nc.scalar.activation(t_g[:, 4 * ch:4 * (ch + 1)],
                                     ps[:, 0:4], EXP)
            sqx.__exit__(None, None, None)

            # ---- Phase 2+3: per head (v_aug emitted after head-0 maps) ----
            actx = tc.tile_pool(name="attn", bufs=1)
            apool = actx.__enter__()
            hctx = (tc.tile_pool(name="headbuf", bufs=2),
                    tc.tile_pool(name="work", bufs=3))
            hpool, wpool = [c.__enter__() for c in hctx]
            t_attnT = [apool.tile([64, L], bf16, name=f"attnT{i}",
                                  tag=f"attnT{i}") for i in range(4)]
            for h in range(HPC):
                hh = h // 2
                hr = slice(64 * (h % 2), 64 * (h % 2) + 64)
                pr = hr
                t_qp = [hpool.tile([128, L], f32r, name=f"qp{i}", tag=f"qp{i}")
                        for i in range(2)]
                t_kp = [hpool.tile([128, L], f32r, name=f"kp{i}", tag=f"kp{i}")
                        for i in range(2)]
                t_kplc = hpool.tile([128, NC2 * 256], bf16, tag="kplc")
                t_S = hpool.tile([128, 132], f32r, tag="S")
                t_rcp = hpool.tile([1, L], f32r, tag="rcp", bufs=1)
                t_raw = hpool.tile([64, L], bf16, tag="raw")
                for lt in range(NLT):
                    ls = slice(lt * LT, (lt + 1) * LT)
                    for half in range(2):
                        mh = slice(128 * half, 128 * (half + 1))
                        ps = psQK_pool.tile([128, LT], f32, tag="psQK")
                        nc.tensor.matmul(ps[:, :], c_projT[pr, mh],
                                         t_qT[hh][hr, ls],
                                         start=True, stop=True)
                        nc.scalar.activation(t_qp[half][:, ls], ps[:, :], EXP)
                        ps2 = psQK_pool.tile([128, LT], f32, tag="psQK")
                        nc.tensor.matmul(ps2[:, :], c_projT[pr, mh],
                                         t_kT[hh][hr, ls],
                                         start=True, stop=True)
                        nc.scalar.activation(t_kp[half][:, ls], ps2[:, :], EXP)
                for j in range(NCC):  # kplc, two 128-chunks per psum tile
                    cs0 = slice(j * 256, j * 256 + 128)
                    cs1 = slice(j * 256 + 128, (j + 1) * 256)
                    ps = psQK_pool.tile([128, 512], f32, tag="psQK")
                    nc.tensor.matmul(ps[:, 0:256], t_kT[hh][hr, cs0],
                                     c_projT[pr, :], start=True, stop=True)
                    nc.tensor.matmul(ps[:, 256:512], t_kT[hh][hr, cs1],
                                     c_projT[pr, :], start=True, stop=True)
                    nc.scalar.activation(
                        t_kplc[:, 512 * j:512 * (j + 1)], ps[:, :], EXP)

                if h == 0:
                    # v_aug = ((Wv x + bv) | ones) * g -> bf16 per-head slots.
                    # Emitted here: its PE/DVE work overlaps head-0's Exps.
                    for ch in range(NC2):
                        cs = slice(ch * 128, (ch + 1) * 128)
                        ps = psA_pool.tile([128, 512], f32, tag="psA")
                        for kt in range(4):
                            nc.tensor.matmul(ps[:, 0:264],
                                             t_x[("v", kt)][:, cs],
                                             c_wv[kt][:, :],
                                             start=(kt == 0), stop=False)
                        nc.tensor.matmul(ps[:, 0:264], c_ones_bf[0:1, 0:128],
                                         c_wvb[:, :], start=False, stop=True)
                        for hv in range(HPC):
                            nc.vector.tensor_scalar_mul(
                                t_v[:, ch * 264 + hv * 66:
                                    ch * 264 + (hv + 1) * 66],
                                ps[:, hv * 66:(hv + 1) * 66],
                                t_g[:, 4 * ch + hv:4 * ch + hv + 1])

                # scan
                nc.gpsimd.tensor_copy(t_S[:, :], c_zero[:, :])
                for cc in range(NCC):
                    qs = slice(cc * CW, (cc + 1) * CW)
                    ts0 = slice(cc * CW, cc * CW + 128)
                    ts1 = slice(cc * CW + 128, (cc + 1) * CW)
                    psA = psA_pool.tile([128, 512], f32, tag="psA")
                    nc.tensor.matmul(psA[:, 0:256], t_kp[0][:, ts0],
                                     t_qp[0][:, qs], start=True, stop=False)
                    nc.tensor.matmul(psA[:, 0:256], t_kp[1][:, ts0],
                                     t_qp[1][:, qs], start=False, stop=True)
                    nc.tensor.matmul(psA[:, 256:512], t_kp[0][:, ts1],
                                     t_qp[0][:, qs], start=True, stop=False)
                    nc.tensor.matmul(psA[:, 256:512], t_kp[1][:, ts1],
                                     t_qp[1][:, qs], start=False, stop=True)
                    atm = wpool.tile([128, 512], bf16, tag="atm")
                    nc.vector.tensor_mul(atm[:, :], psA[:, :], c_msk[:, :])
                    if cc % 2 == 0:
                        nd2 = psND_pool.tile([66, 2 * CW], f32, tag="psNDt")
                    nd = nd2[:, (cc % 2) * CW:(cc % 2 + 1) * CW]
                    c128 = cc * 2
                    va0 = t_v[:, c128 * 264 + h * 66:c128 * 264 + h * 66 + 66]
                    va1 = t_v[:, (c128 + 1) * 264 + h * 66:
                              (c128 + 1) * 264 + h * 66 + 66]
                    nc.tensor.matmul(nd[:, :], va0, atm[:, 0:256],
                                     start=True, stop=False)
                    nc.tensor.matmul(nd[:, :], va1, atm[:, 256:512],
                                     start=False, stop=(cc == 0))
                    if cc > 0:
                        nc.tensor.matmul(nd[:, :], t_S[:, 0:66],
                                         t_qp[0][:, qs],
                                         start=False, stop=False)
                        nc.tensor.matmul(nd[:, :], t_S[:, 66:132],
                                         t_qp[1][:, qs],
                                         start=False, stop=True)
                    if cc % 2 == 1:
                        ds = slice((cc - 1) * CW, (cc + 1) * CW)
                        with nc.allow_low_precision(reason="f32r reciprocal for matmul broadcast (validated vs reference)"):
                            nc.vector.reciprocal(t_rcp[0:1, ds],
                                                 nd2[64:65, :])
                        if cc % 4 == 1:
                            nc.scalar.copy(t_raw[:, ds], nd2[0:64, :])
                        else:
                            nc.vector.tensor_copy(t_raw[:, ds], nd2[0:64, :])
                    if cc < NCC - 1:
                        psS = psS_pool.tile([128, 132], f32, tag="psS")
                        nc.tensor.matmul(
                            psS[:, 0:66],
                            t_kplc[:, c128 * 256:c128 * 256 + 128],
                            va0, start=True, stop=False)
                        nc.tensor.matmul(
                            psS[:, 0:66],
                            t_kplc[:, (c128 + 1) * 256:(c128 + 1) * 256 + 128],
                            va1, start=False, stop=True)
                        nc.tensor.matmul(
                            psS[:, 66:132],
                            t_kplc[:, c128 * 256 + 128:c128 * 256 + 256],
                            va0, start=True, stop=False)
                        nc.tensor.matmul(
                            psS[:, 66:132],
                            t_kplc[:, (c128 + 1) * 256 + 128:(c128 + 2) * 256],
                            va1, start=False, stop=True)
                        with nc.allow_low_precision(reason="f32r state accumulate (TF32-rounding ~1e-3, validated vs reference)"):
                            nc.vector.tensor_add(t_S[:, :], t_S[:, :],
                                                 psS[:, :])
                # division for the whole head, off the chunk chain
                for lt in range(NLT):
                    ls = slice(lt * LT, (lt + 1) * LT)
                    psB = psB_pool.tile([64, LT], f32, tag="psB")
                    nc.tensor.matmul(psB[:, :], c_ones[0:1, 0:64],
                                     t_rcp[0:1, ls], start=True, stop=True)
                    nc.vector.tensor_mul(t_attnT[h][:, ls], t_raw[:, ls],
                                         psB[:, :])
            for c in reversed(hctx):
                c.__exit__(None, None, None)

            # ---- Phase 4: output projection ----
            octx = tc.tile_pool(name="outp", bufs=4)
            opool = octx.__enter__()
            for lt in range(NLT):
                ls = slice(lt * LT, (lt + 1) * LT)
                for osub in range(4):
                    os_ = slice(128 * osub, 128 * (osub + 1))
                    ps = psA_pool.tile([128, 512], f32, tag="psA")
                    for h in range(4):
                        nc.tensor.matmul(ps[:, 0:LT], c_wo[h][:, os_],
                                         t_attnT[h][:, ls],
                                         start=(h == 0), stop=(h == 3))
                    t_o = opool.tile([128, LT], bf16, tag="outT")
                    nc.scalar.copy(t_o[:, :], ps[:, 0:LT])
                    nc.sync.dma_start(outT[os_, ls], t_o[:, :])
            octx.__exit__(None, None, None)
            actx.__exit__(None, None, None)
            xvin.__exit__(None, None, None)

    nc.compile()
    return nc


def _prep_inputs(query, key, value, Wq, bq, Wk, bk, Wv, bv, Wo, bo, proj):
    import ml_dtypes
    s = float(DK) ** -0.25
    tri = (np.arange(128)[:, None] <= np.arange(128)[None, :]).astype(np.float32)
    on = np.ones((128, 128), np.float32)
    zr = np.zeros((128, 128), np.float32)
    msk = np.concatenate([tri, on, zr, tri], axis=1)
    bd = np.zeros((128, 8), np.float32)
    for half in range(2):
        for r in range(128):
            bd[r, 4 * half + (2 * half + r // 64)] = -0.5
    pT = np.ascontiguousarray(proj.T)
    common = {"projT": np.concatenate([pT, pT]), "bd": bd, "msk": msk,
              "ones": np.ones((1, 128), np.float32),
              "ones_bf": np.ones((1, 128), ml_dtypes.bfloat16)}
    in_maps = []
    for b in range(B):
        for hg in range(2):
            sl = slice(hg * 256, (hg + 1) * 256)
            Wqs, Wks, Wvs = Wq[sl] * s, Wk[sl] * s, Wv[sl]
            bqs, bks, bvs = bq[sl] * s, bk[sl] * s, bv[sl]
            # wv_aug [513, 264]: per head h cols 66h:66h+64 = Wv.T head cols,
            # cols 66h+64:66h+66 zero; bias row: bv at head cols, 1.0 at ones
            wv_aug = np.zeros((513, 264), np.float32)
            for h in range(HPC):
                wv_aug[0:512, 66 * h:66 * h + 64] = Wvs.T[:, 64 * h:64 * (h + 1)]
                wv_aug[512, 66 * h:66 * h + 64] = bvs[64 * h:64 * (h + 1)]
                wv_aug[512, 66 * h + 64:66 * h + 66] = 1.0
            m = dict(common)
            m["xq"] = np.ascontiguousarray(query[b].T)
            m["xk"] = np.ascontiguousarray(key[b].T)
            m["xv"] = np.ascontiguousarray(value[b].T)
            m["wq"] = np.ascontiguousarray(Wqs.T)
            m["wk"] = np.ascontiguousarray(Wks.T)
            m["wv"] = wv_aug
            m["bq2"] = np.stack([bqs[:128], bqs[128:]], axis=1)
            m["bk2"] = np.stack([bks[:128], bks[128:]], axis=1)
            mm = {k: (np.ascontiguousarray(v) if v.dtype == ml_dtypes.bfloat16
                      else np.ascontiguousarray(v, np.float32))
                  for k, v in m.items()}
            for k in ("xq", "xk", "xv", "wq", "wk", "wv"):
                mm[k] = mm[k].astype(ml_dtypes.bfloat16)
            mm["wo"] = np.ascontiguousarray(Wo[:, sl].T).astype(
                ml_dtypes.bfloat16)
            in_maps.append(mm)
    return in_maps


def kernel(query, key, value, Wq, bq, Wk, bk, Wv, bv, Wo, bo, proj,
           _trace=False):
    global _COMPILED
    from concourse import bass_utils
    args = [np.asarray(a, np.float32) for a in
            (query, key, value, Wq, bq, Wk, bk, Wv, bv, Wo, bo, proj)]
    if _COMPILED is None:
        _COMPILED = _build()
    in_maps = _prep_inputs(*args)
    res = bass_utils.run_bass_kernel_spmd(
        _COMPILED, in_maps, core_ids=list(range(8)), trace=_trace)
    out = np.empty((B, L, DIM), np.float32)
    bo_ = args[10]
    for b in range(B):
        out[b] = (res.results[2 * b]["outT"].T.astype(np.float32)
                  + res.results[2 * b + 1]["outT"].T.astype(np.float32)
                  + bo_)
    if _trace:
        kernel._last = res
    return out
